# revision 53
# baseline (speedup 1.0000x reference)
"""Trainium2 Bass/Tile kernel: fused fp8-quantized multi-head causal attention.

Module: q/k/v = fp8(x) @ fp8(W) + b ; scores = (q k^T)/sqrt(64) with causal
mask (-1000 => exp underflows to exactly 0) ; out = softmax(scores) @ v @ W_O + b_O.

Sharding (8 NeuronCores, SPMD, no collectives):
  core c -> batch b = c // 4, head group hg = c % 4 (heads 4*hg .. 4*hg+3).
  Each core returns a partial [S, M] bf16 output (its 4 heads' contribution);
  the host sums the 4 partials per batch (fp32) and adds b_O.

Host-side preprocessing: inputs/W_{Q,K,V} are quantized to fp8-e4m3 on the
host (bit-identical to the reference's jnp e4m3fn cast for |x| <= 240) and
uploaded in the exact SBUF layout so every DMA is fully contiguous.
Startup-latency layout choices (the head is DMA-bandwidth-bound):
  * W_Q/W_K are packed [128, 2 c-halves, 8 m-chunks, 128] and DMA'd per
    c-half (128 KiB), so the first q/k projections wait only on their own
    half; W_V keeps the [128, 8, 256] layout (the v-proj rhs needs a 3D AP).
  * x_v is packed subchunk-major [128, 16 s-subchunks, 8 m-chunks, 128] so
    the first v projection gates on a 256 KiB slice instead of a full
    512 KiB window; xq/xk window-0 DMAs are split into two m-halves so the
    projection matmuls pipeline behind the transfers (view-level deps).
  This moves the first real matmul from ~18 us to ~14 us after kernel start.

On-chip dataflow per core (single fused loop, paced by ScalarE exp and the
PE roughly equally):
  qT, kT   : [d'=256, S] bf16, fp8 DoubleRow matmuls; bias added in the DVE
             psum eviction; the 1/8 score scale is folded into the kT
             eviction (fused add+mult tensor_scalar).
  v        : [S, 4x(64+ones-col)] bf16 (ones col accumulates the softmax
             denominator in z's row 64 for free); b_V is added by the DVE
             eviction against a gpsimd-broadcast bias tile (no fp32 matmuls).
  scores^T : one 2-bank psum [sk=128, 2, sq=512] per sk-chunk holds BOTH
             heads of an even/odd pair (row-group-packed concurrent matmuls,
             K=64 each). Diagonal-band tiles compute only the causally-live
             column range. pps bufs=3 lets scores run two exps ahead.
  pattern  : ONE exp per sk-chunk on ScalarE over both heads (2x512-wide)
             -> bf16; the diagonal [128,2,128] triangular mask is applied
             in-place by a single gpsimd affine_select covering both heads.
  z^T+denom: psum [65, sq] += v_h(lhsT [sk,65]) @ pattern, software-pipelined
             one sk-chunk BEHIND the scores/exp so the PE never stalls on
             the scalar engine. On band chunks the causally-full columns are
             a separate matmul that does not wait for the gpsimd mask.
  normalize: 1/denom via reciprocal_approx_fast, gpsimd partition_broadcast,
             DVE multiply -> bf16 zT.
  out      : psum [s=128, m=512] = zt(lhsT [hd=128,s])^T @ W_O; evicted to
             bf16 ([128,512] halves) and DMA'd per half.
  Scheduling: the qT/kT/v projections and the previous window's output
  projection are emitted as deadline-driven filler units inside the
  attention loop (one per sk-chunk slot), so DMA-in, projections, attention
  and DMA-out all overlap. A block of 36 tiny matmuls right after the DMA
  issues keeps the PE HAM activity monitor warm so real matmuls start at
  2.4 GHz; a few wide (N=512) warmups after them bridge the remaining
  DMA-bound gap so the HAM never re-throttles before the first projection.
  The last window's normalize+output-projection tail is
  fine-grained per 128-row slice with psum evictions moved to the by-then
  idle scalar engine.
"""

import os
import sys

for _p in ("/opt/trn_rl_repo", os.path.expanduser("~/.axon_site/_ro/trn_rl_repo")):
    if os.path.isdir(_p) and _p not in sys.path:
        sys.path.insert(0, _p)

import ml_dtypes
import numpy as np

import concourse.bass as bass
import concourse.mybir as mybir
import concourse.tile as tile
from concourse import bacc
from concourse.bass_utils import run_bass_kernel_spmd

B, S, M, H, D = 2, 2048, 1024, 16, 64
HG = 4                 # heads per core
NCORES = 8
SQ = 512               # sq chunk width (one fp32 psum bank)
NSQ = S // SQ          # 4
NMC = M // 128         # 8 contraction chunks for projections
NSS = S // 128         # 16 s sub-chunks of 128

F8 = mybir.dt.float8e4
BF = mybir.dt.bfloat16
F32 = mybir.dt.float32
EXP = mybir.ActivationFunctionType.Exp
DR = mybir.MatmulPerfMode.DoubleRow

_f8 = ml_dtypes.float8_e4m3
_bf16 = ml_dtypes.bfloat16

# Schraudolph exp as a bf16 bit pattern: uint16(x * 128/ln2 + 16250)
# (~±3% relative; used on the DVE for half the last window's chunks, where
# the kernel is otherwise ScalarE-ACTIVATE bound)
EXPA = 184.6650390625
EXPB = 16250.0
U16 = mybir.dt.uint16


def _build_nc():
    nc = bacc.Bacc(
        "TRN2", target_bir_lowering=False, debug=False, num_devices=NCORES
    )

    xq = nc.declare_dram_parameter("xq_t8", [128, NSQ, NMC, SQ], F8, isOutput=False)
    xk = nc.declare_dram_parameter("xk_t8", [128, NSQ, NMC, SQ], F8, isOutput=False)
    xv = nc.declare_dram_parameter("xv_t8", [128, NSS, NMC, 128], F8, isOutput=False)
    wqd = nc.declare_dram_parameter("wq8", [128, 2 * NMC * 128], F8, isOutput=False)
    wkd = nc.declare_dram_parameter("wk8", [128, 2 * NMC * 128], F8, isOutput=False)
    wvd = nc.declare_dram_parameter("wv8", [128, NMC * HG * D], F8, isOutput=False)
    wo = nc.declare_dram_parameter("wo_bf", [128, 2 * M], BF, isOutput=False)
    bqk = nc.declare_dram_parameter("bqk", [128, 4], F32, isOutput=False)
    bv = nc.declare_dram_parameter("bv", [1, HG * D], F32, isOutput=False)
    out_p = nc.declare_dram_parameter("out_p", [S, M], BF, isOutput=True)

    with tile.TileContext(nc) as tc:
        with (
            tc.tile_pool(name="persist", bufs=1) as pers,
            tc.tile_pool(name="work", bufs=2) as work,
            tc.tile_pool(name="pps", bufs=3, space="PSUM") as pps,
            tc.tile_pool(name="ppz", bufs=2, space="PSUM") as ppz,
        ):
            # ---- persistent SBUF tensors ----
            xq_sb = pers.tile([128, NSQ, NMC, SQ], F8, tag="xq")
            xk_sb = pers.tile([128, NSQ, NMC, SQ], F8, tag="xk")
            xv_sb = pers.tile([128, NSS, NMC, 128], F8, tag="xv")
            wq_c = [pers.tile([128, NMC, 128], F8, tag=f"wq{c}",
                                 name=f"wq_c{c}") for c in range(2)]
            wk_c = [pers.tile([128, NMC, 128], F8, tag=f"wk{c}",
                                 name=f"wk_c{c}") for c in range(2)]
            wv_sb = pers.tile([128, NMC, HG * D], F8, tag="wv")
            wo_sb = pers.tile([128, 2, M], BF, tag="wo")
            bqk_sb = pers.tile([128, 4], F32, tag="bqk")
            bv_sb = pers.tile([1, HG * D], F32, tag="bv")
            bvb_sb = pers.tile([128, HG, D], F32, tag="bvb")
            qt_sb = pers.tile([128, 2, S], BF, tag="qt")
            kt_sb = pers.tile([128, 2, S], BF, tag="kt")
            zt_sb = pers.tile([128, 2, S], BF, tag="zt")
            v_sb = pers.tile([128, NSS, HG, D + 1], BF, tag="v")
            expwarm = pers.tile([1, 1], F32, tag="expwarm")

            # ---- input DMAs (issued before anything that could block the
            # issuing queues), then constants / exp-table warm-up ----
            wq4 = wqd[:, :].rearrange("p (c m d) -> p c m d", c=2, m=NMC)
            wk4 = wkd[:, :].rearrange("p (c m d) -> p c m d", c=2, m=NMC)
            wv4 = wvd[:, :].rearrange("p (m d) -> p m d", m=NMC)
            nc.scalar.dma_start(out=bqk_sb[:, :], in_=bqk[:, :])
            nc.scalar.dma_start(out=bv_sb[:, :], in_=bv[:, :])
            nc.scalar.dma_start(out=wq_c[0][:, :, :], in_=wq4[:, 0])
            nc.sync.dma_start(out=xq_sb[:, 0, 0:4], in_=xq[:, 0, 0:4])
            nc.scalar.dma_start(out=wk_c[0][:, :, :], in_=wk4[:, 0])
            nc.sync.dma_start(out=xq_sb[:, 0, 4:8], in_=xq[:, 0, 4:8])
            nc.scalar.dma_start(out=wv_sb[:, :, :], in_=wv4)
            nc.sync.dma_start(out=xk_sb[:, 0, 0:4], in_=xk[:, 0, 0:4])
            nc.scalar.dma_start(out=wq_c[1][:, :, :], in_=wq4[:, 1])
            nc.sync.dma_start(out=xk_sb[:, 0, 4:8], in_=xk[:, 0, 4:8])
            nc.scalar.dma_start(out=wk_c[1][:, :, :], in_=wk4[:, 1])
            nc.sync.dma_start(out=xv_sb[:, 0:2], in_=xv[:, 0:2])
            nc.sync.dma_start(out=xv_sb[:, 2:4], in_=xv[:, 2:4])
            for t in range(1, NSQ):
                nc.sync.dma_start(out=xq_sb[:, t], in_=xq[:, t])
                nc.sync.dma_start(out=xk_sb[:, t], in_=xk[:, t])
                nc.sync.dma_start(out=xv_sb[:, 4 * t : 4 * t + 4],
                                  in_=xv[:, 4 * t : 4 * t + 4])
            nc.scalar.dma_start(
                out=wo_sb[:, :, :], in_=wo[:, :].rearrange("p (c m) -> p c m", c=2)
            )

            # warm the exp table set while the DMAs run
            nc.gpsimd.memset(expwarm[:, :], 1.0)
            nc.gpsimd.memset(v_sb[:, :, :, D : D + 1], 1.0)
            nc.scalar.activation(expwarm[:, :], expwarm[:, :], EXP)

            # keep the PE HAM activity monitor busy while the first DMAs
            # land so the real matmuls start at the warm (2.4 GHz) clock
            warm_bf = pers.tile([1, 64], BF, tag="warm_bf")
            nc.gpsimd.memset(warm_bf[:, :], 1.0)
            warm_w = pers.tile([1, SQ], BF, tag="warm_w")
            nc.gpsimd.memset(warm_w[:, :], 1.0)

            # broadcast b_V across partitions for the v-eviction add
            nc.gpsimd.partition_broadcast(
                bvb_sb.rearrange("p g d -> p (g d)"), bv_sb[0:1, :]
            )

            ps_w2 = pps.tile([128, 2, SQ], F32, tag="pps", name="ps_warm")
            for _ in range(36):
                nc.tensor.matmul(
                    ps_w2[0:64, 0, 0:64],
                    lhsT=warm_bf[0:1, :],
                    rhs=warm_bf[0:1, :],
                    start=True,
                    stop=True,
                )
            # a few wide warmups extend PE activity past the DMA-bound gap
            # before the first projection, so the HAM MID window (~3.4 us of
            # idle) never re-throttles the clock to 1.2 GHz before real work
            for _ in range(6):
                nc.tensor.matmul(
                    ps_w2[0:64, 0, :],
                    lhsT=warm_bf[0:1, :],
                    rhs=warm_w[0:1, :],
                    start=True,
                    stop=True,
                )

            # ---- filler units (emitted inside the attention loop) ----
            def projqk_unit(dst_sb, w_sb, x_sb, bcol, t, c, scale=None):
                """qT/kT projection for s-window t, head-pair half c."""

                def emit():
                    ssl = slice(SQ * t, SQ * t + SQ)
                    ps2 = pps.tile([128, 2, SQ], F32, tag="pps", name=f"pj{t}_{c}")
                    ps = ps2[:, 0, :]
                    for mi in range(0, NMC, 2):
                        nc.tensor.matmul(
                            ps[:, :],
                            lhsT=w_sb[c][:, mi : mi + 2, :],
                            rhs=x_sb[:, t, mi : mi + 2, :],
                            start=(mi == 0),
                            stop=(mi == NMC - 2),
                            perf_mode=DR,
                        )
                    if scale is None:
                        nc.vector.tensor_scalar_add(
                            dst_sb[:, c, ssl], ps[:, :], bqk_sb[:, bcol : bcol + 1]
                        )
                    else:
                        # fold the 1/8 attention scale into the kT eviction
                        nc.vector.tensor_scalar(
                            out=dst_sb[:, c, ssl],
                            in0=ps[:, :],
                            scalar1=bqk_sb[:, bcol : bcol + 1],
                            scalar2=scale,
                            op0=mybir.AluOpType.add,
                            op1=mybir.AluOpType.mult,
                        )

                return emit

            def projv_unit(ss):
                """v projection for s-subchunk ss (128 rows)."""

                def emit():
                    psl = slice(128 * ss, 128 * ss + 128)
                    ps2 = pps.tile([128, 2, SQ], F32, tag="pps", name=f"pv{ss}")
                    ps = ps2[:, 0, :]
                    for mi in range(0, NMC, 2):
                        nc.tensor.matmul(
                            ps[:, 0 : HG * D],
                            lhsT=xv_sb[:, ss, mi : mi + 2, :],
                            rhs=wv_sb[:, mi : mi + 2, :],
                            start=(mi == 0),
                            stop=(mi == NMC - 2),
                            perf_mode=DR,
                        )
                    nc.vector.tensor_tensor(
                        out=v_sb[:, ss, :, 0:D],
                        in0=ps[:, 0 : HG * D].rearrange("p (g d) -> p g d", g=HG),
                        in1=bvb_sb[:, :, :],
                        op=mybir.AluOpType.add,
                    )

                return emit

            def outproj_unit(jq, ss4, n, evict=None):
                """output projection for rows [512*jq + 128*ss4, +128),
                columns [512*n, +512)."""

                def emit():
                    psl = slice(SQ * jq + 128 * ss4, SQ * jq + 128 * ss4 + 128)
                    nsl = slice(SQ * n, SQ * n + SQ)
                    o16 = work.tile([128, SQ], BF, tag="o16", bufs=3,
                                    name=f"o{jq}_{ss4}_{n}")
                    ps_o2 = pps.tile([128, 2, SQ], F32, tag="pps",
                                     name=f"po{jq}_{ss4}_{n}")
                    ps_o = ps_o2[:, 0, :]
                    for c in range(2):
                        nc.tensor.matmul(
                            ps_o[:, :],
                            lhsT=zt_sb[:, c, psl],
                            rhs=wo_sb[:, c, nsl],
                            start=(c == 0),
                            stop=(c == 1),
                        )
                    if evict == "scalar":
                        nc.scalar.copy(o16[:, :], ps_o[:, :])
                    else:
                        nc.vector.tensor_copy(o16[:, :], ps_o[:, :])
                    nc.sync.dma_start(out=out_p[psl, nsl], in_=o16[:, :])

                return emit

            def emit_z(ps_z, c, prev, jq, last, only_u=None):
                """z += v.T @ pattern for one sk-chunk. On diagonal-band
                chunks the causally-full columns go first (they do not wait
                for the gpsimd mask); the masked diagonal block follows."""
                pp, psi, pw0 = prev
                band = psi >= 4 * jq
                ranges = []
                if band and pw0 + 128 < SQ:
                    ranges.append((pw0 + 128, SQ, False))
                if band:
                    ranges.append((pw0, pw0 + 128, True))
                if not band:
                    ranges.append((pw0, SQ, False))
                for u in range(2) if only_u is None else (only_u,):
                    for ri, (lo, hi, _) in enumerate(ranges):
                        nc.tensor.matmul(
                            ps_z[u][:, lo:hi],
                            lhsT=v_sb[:, psi, 2 * c + u, :],
                            rhs=pp[:, u, lo:hi],
                            start=(psi == 0 and ri == 0),
                            stop=(last and ri == len(ranges) - 1),
                        )

            # ---- fused schedule ----
            # minimal pre-loop: just q/k window 0 half 0 before jq=0
            projqk_unit(qt_sb, wq_c, xq_sb, 0, 0, 0)()
            projqk_unit(kt_sb, wk_c, xk_sb, 2, 0, 0, 0.125)()

            for jq in range(NSQ):
                qsl = slice(SQ * jq, SQ * jq + SQ)
                nsk = 4 * (jq + 1)
                # deadline-driven filler assignment:
                #   c0 slot i (i<4): v chunk 4*jq+i (first used by z at
                #     absolute chunk 4*jq+i, i.e. c0 slot 4*jq+i+1)
                #   c0 later slots + c1 slots: next window's q/k and the
                #     previous window's output projection
                fill = {0: {}, 1: {}}
                for i in range(4):
                    fill[0].setdefault(i, []).append(projv_unit(4 * jq + i))
                if jq == 0:
                    fill[0].setdefault(1, []).append(
                        projqk_unit(qt_sb, wq_c, xq_sb, 1, 0, 1))
                    fill[0].setdefault(2, []).append(
                        projqk_unit(kt_sb, wk_c, xk_sb, 3, 0, 1, 0.125))
                rest0, rest1 = [], []
                if jq > 0:
                    oph = [outproj_unit(jq - 1, ss4, n)
                           for ss4 in range(4) for n in range(2)]
                    rest0 += oph[:4]
                    rest1 += oph[4:]
                if jq < NSQ - 1:
                    t = jq + 1
                    rest1 = [
                        projqk_unit(qt_sb, wq_c, xq_sb, 0, t, 0),
                        projqk_unit(kt_sb, wk_c, xk_sb, 2, t, 0, 0.125),
                        projqk_unit(qt_sb, wq_c, xq_sb, 1, t, 1),
                        projqk_unit(kt_sb, wk_c, xk_sb, 3, t, 1, 0.125),
                    ] + rest1
                for lst, cc in ((rest0, 0), (rest1, 1)):
                    free = nsk - 4 if cc == 0 else nsk
                    base = 4 if cc == 0 else 0
                    for i, u in enumerate(lst):
                        s = base + (i * free // len(lst) if free > 0 else i)
                        fill[cc].setdefault(min(s, max(nsk - 2, 0)), []).append(u)

                for c in range(2):  # head pair: heads (2c, 2c+1)
                    ps_z = [
                        ppz.tile([D + 1, SQ], F32, tag="ppz", name=f"psz{jq}_{c}_{u}")
                        for u in range(2)
                    ]
                    prev = None  # delayed-z pipeline: (p_bf, si, w0)
                    for si in range(nsk):
                        ksl = slice(128 * si, 128 * si + 128)
                        r = si - 4 * jq  # >=0 on diagonal-band tiles
                        w0 = 128 * r if r > 0 else 0  # fully-masked prefix
                        # both heads' scores into one 2-bank psum tile
                        ps2 = pps.tile([128, 2, SQ], F32, tag="pps",
                                       name=f"ps{jq}_{c}_{si}")
                        for u in range(2):
                            hsl = slice(64 * u, 64 * u + 64)
                            nc.tensor.matmul(
                                ps2[:, u, w0:SQ],
                                lhsT=kt_sb[hsl, c, ksl],
                                rhs=qt_sb[hsl, c, SQ * jq + w0 : SQ * jq + SQ],
                                start=True,
                                stop=True,
                            )
                        p_bf = work.tile([128, 2, SQ], BF, tag="p", bufs=4,
                                         name=f"p{jq}_{c}_{si}")
                        nc.scalar.activation(
                            p_bf[:, :, w0:SQ], ps2[:, :, w0:SQ], EXP
                        )
                        if r >= 0:
                            # in-place triangular mask on the diagonal block,
                            # both heads in one gpsimd op: keep col >= row
                            nc.gpsimd.affine_select(
                                out=p_bf[:, :, w0 : w0 + 128],
                                in_=p_bf[:, :, w0 : w0 + 128],
                                compare_op=mybir.AluOpType.is_ge,
                                fill=0.0,
                                base=0,
                                pattern=[[0, 2], [1, 128]],
                                channel_multiplier=-1,
                            )
                        if prev is not None:
                            emit_z(ps_z, c, prev, jq, last=False)
                        prev = (p_bf, si, w0)
                        for emit in fill[c].get(si, ()):
                            emit()
                    # drain the pipelined z for the last sk-chunk
                    emit_z(ps_z, c, prev, jq, last=True)
                    # normalize both heads of the pair.  The z psum is freed
                    # as fast as possible: the denominator row goes out via a
                    # ScalarE copy while a DVE copy evicts the raw z rows; the
                    # recip/broadcast/multiply then run off the critical path.
                    recips, rbs, zraws = [], [], []
                    dns = []
                    for u in range(2):
                        dn = work.tile([1, SQ], F32, tag="dn", name=f"dn{jq}{c}{u}")
                        nc.vector.tensor_copy(dn[:, :], ps_z[u][D : D + 1, :])
                        dns.append(dn)
                    for u in range(2):
                        recip = work.tile([1, SQ], F32, tag="recip",
                                          name=f"rc{jq}{c}{u}")
                        nc.vector.reciprocal_approx_fast(
                            out=recip[:, :], in_=dns[u][:, :]
                        )
                        recips.append(recip)
                    for u in range(2):
                        rb = work.tile([D, SQ], F32, tag="rb", name=f"rb{jq}{c}{u}")
                        nc.gpsimd.partition_broadcast(rb[:, :], recips[u][0:1, :])
                        rbs.append(rb)
                    if jq == NSQ - 1 and c == 1:
                        # fine-grained tail: normalize per 128-row slice and
                        # start that slice's output projection immediately
                        for ss4 in range(4):
                            fsl = slice(128 * ss4, 128 * ss4 + 128)
                            for u in range(2):
                                nc.vector.tensor_mul(
                                    zt_sb[64 * u : 64 * u + 64, c,
                                          SQ * jq + 128 * ss4 : SQ * jq + 128 * ss4 + 128],
                                    ps_z[u][0:D, fsl],
                                    rbs[u][:, fsl],
                                )
                            outproj_unit(jq, ss4, 0, evict="scalar")()
                            outproj_unit(jq, ss4, 1)()
                    else:
                        for u in range(2):
                            nc.vector.tensor_mul(
                                zt_sb[64 * u : 64 * u + 64, c, qsl],
                                ps_z[u][0:D, :],
                                rbs[u][:, :],
                            )

    if not nc.is_finalized():
        nc.finalize()
    return nc


_NC = None


def _get_nc():
    global _NC
    if _NC is None:
        _NC = _build_nc()
    return _NC


def _wpack(w):
    """[M, HG*D] -> partition-major [128, NMC*HG*D] (2 KiB contiguous rows)."""
    return np.ascontiguousarray(
        w.reshape(NMC, 128, HG * D).transpose(1, 0, 2).reshape(128, NMC * HG * D)
    )


def _wpack_c(w):
    """[M, HG*D] -> [128, 2(c-half), NMC, 128] flattened (c-half contiguous)."""
    return np.ascontiguousarray(
        w.reshape(NMC, 128, 2, 128).transpose(1, 2, 0, 3).reshape(128, 2 * NMC * 128)
    )


def _make_in_maps(inputs):
    q8 = lambda a: np.asarray(a, np.float32).astype(_f8)
    xt = {}
    for name, key in (("xq_t8", "query_input"), ("xk_t8", "key_input"),
                      ("xv_t8", "value_input")):
        # [S, M] -> fp8 [M, S] -> [p=128, t=4, mi=8, s'=512] (SBUF layout)
        if name == "xv_t8":
            # subchunk-major: [p, ss=16, mi=8, 128] so the v projection for
            # s-subchunk ss gates on a 128 KiB slice instead of a full window
            xt[name] = [
                np.ascontiguousarray(
                    q8(inputs[key][b]).T.reshape(NMC, 128, NSS, 128)
                    .transpose(1, 2, 0, 3)
                )
                for b in range(B)
            ]
        else:
            xt[name] = [
                np.ascontiguousarray(
                    q8(inputs[key][b]).T.reshape(NMC, 128, NSQ, SQ)
                    .transpose(1, 2, 0, 3)
                )
                for b in range(B)
            ]

    wq8 = q8(inputs["W_Q"])  # [H, M, D]
    wk8 = q8(inputs["W_K"])
    wv8 = q8(inputs["W_V"])
    wo = np.asarray(inputs["W_O"], np.float32)  # [H, D, M]

    in_maps = []
    for core in range(NCORES):
        b, hg = core // HG, core % HG
        hs = slice(HG * hg, HG * hg + HG)
        m = {
            "xq_t8": xt["xq_t8"][b],
            "xk_t8": xt["xk_t8"][b],
            "xv_t8": xt["xv_t8"][b],
            "wq8": _wpack_c(wq8[hs].transpose(1, 0, 2).reshape(M, HG * D)),
            "wk8": _wpack_c(wk8[hs].transpose(1, 0, 2).reshape(M, HG * D)),
            "wv8": _wpack(wv8[hs].transpose(1, 0, 2).reshape(M, HG * D)),
            "wo_bf": np.ascontiguousarray(
                wo[hs]
                .reshape(HG * D, M)
                .astype(_bf16)
                .reshape(2, 128, M)
                .transpose(1, 0, 2)
                .reshape(128, 2 * M)
            ),
            "bqk": np.ascontiguousarray(
                np.concatenate(
                    [
                        np.asarray(inputs[k], np.float32)[hs].reshape(2, 128).T
                        for k in ("b_Q", "b_K")
                    ],
                    axis=1,
                )
            ),
            "bv": np.asarray(inputs["b_V"], np.float32)[hs].reshape(1, HG * D).copy(),
        }
        in_maps.append(m)
    return in_maps


def _run(inputs, **kw):
    nc = _get_nc()
    in_maps = _make_in_maps(inputs)
    res = run_bass_kernel_spmd(nc, in_maps, list(range(NCORES)), **kw)
    out = np.zeros((B, S, M), np.float32)
    for core in range(NCORES):
        out[core // HG] += res.results[core]["out_p"].astype(np.float32)
    out += np.asarray(inputs["b_O"], np.float32)
    return out, res


def kernel(**inputs):
    out, _ = _run(inputs)
    return out



# revision 54
# speedup vs baseline: 1.0055x; 1.0055x over previous
"""Trainium2 Bass/Tile kernel: fused fp8-quantized multi-head causal attention.

Module: q/k/v = fp8(x) @ fp8(W) + b ; scores = (q k^T)/sqrt(64) with causal
mask (-1000 => exp underflows to exactly 0) ; out = softmax(scores) @ v @ W_O + b_O.

Sharding (8 NeuronCores, SPMD, no collectives):
  core c -> batch b = c // 4, head group hg = c % 4 (heads 4*hg .. 4*hg+3).
  Each core returns a partial [S, M] bf16 output (its 4 heads' contribution);
  the host sums the 4 partials per batch (fp32) and adds b_O.

Host-side preprocessing: inputs/W_{Q,K,V} are quantized to fp8-e4m3 on the
host (bit-identical to the reference's jnp e4m3fn cast for |x| <= 240) and
uploaded in the exact SBUF layout so every DMA is fully contiguous.
Startup-latency layout choices (the head is DMA-bandwidth-bound):
  * W_Q/W_K are packed [128, 2 c-halves, 8 m-chunks, 128] and DMA'd per
    c-half (128 KiB), so the first q/k projections wait only on their own
    half; W_V keeps the [128, 8, 256] layout (the v-proj rhs needs a 3D AP).
  * x_v is packed subchunk-major [128, 16 s-subchunks, 8 m-chunks, 128] so
    the first v projection gates on a 256 KiB slice instead of a full
    512 KiB window; xq/xk window-0 DMAs are split into two m-halves so the
    projection matmuls pipeline behind the transfers (view-level deps).
  This moves the first real matmul from ~18 us to ~14 us after kernel start.

On-chip dataflow per core (single fused loop, paced by ScalarE exp and the
PE roughly equally):
  qT, kT   : [d'=256, S] bf16, fp8 DoubleRow matmuls; bias added in the DVE
             psum eviction; the 1/8 score scale is folded into the kT
             eviction (fused add+mult tensor_scalar).
  v        : [S, 4x(64+ones-col)] bf16 (ones col accumulates the softmax
             denominator in z's row 64 for free); b_V is added by the DVE
             eviction against a gpsimd-broadcast bias tile (no fp32 matmuls).
  scores^T : one 2-bank psum [sk=128, 2, sq=512] per sk-chunk holds BOTH
             heads of an even/odd pair (row-group-packed concurrent matmuls,
             K=64 each). Diagonal-band tiles compute only the causally-live
             column range. pps bufs=3 lets scores run two exps ahead.
  pattern  : ONE exp per sk-chunk on ScalarE over both heads (2x512-wide)
             -> bf16; the diagonal [128,2,128] triangular mask is applied
             in-place by a single gpsimd affine_select covering both heads.
  z^T+denom: psum [65, sq] += v_h(lhsT [sk,65]) @ pattern, software-pipelined
             one sk-chunk BEHIND the scores/exp so the PE never stalls on
             the scalar engine. On band chunks the causally-full columns are
             a separate matmul that does not wait for the gpsimd mask.
  normalize: 1/denom via reciprocal_approx_fast, gpsimd partition_broadcast,
             DVE multiply -> bf16 zT.
  out      : psum [s=128, m=512] = zt(lhsT [hd=128,s])^T @ W_O; evicted to
             bf16 ([128,512] halves) and DMA'd per half.
  Scheduling: the qT/kT/v projections and the previous window's output
  projection are emitted as deadline-driven filler units inside the
  attention loop (one per sk-chunk slot), so DMA-in, projections, attention
  and DMA-out all overlap. A block of 36 tiny matmuls right after the DMA
  issues keeps the PE HAM activity monitor warm so real matmuls start at
  2.4 GHz (36 is tuned: more delays the first projection). The last window's normalize+output-projection tail is
  fine-grained per 128-row slice with psum evictions moved to the by-then
  idle scalar engine.
"""

import os
import sys

for _p in ("/opt/trn_rl_repo", os.path.expanduser("~/.axon_site/_ro/trn_rl_repo")):
    if os.path.isdir(_p) and _p not in sys.path:
        sys.path.insert(0, _p)

import ml_dtypes
import numpy as np

import concourse.bass as bass
import concourse.mybir as mybir
import concourse.tile as tile
from concourse import bacc
from concourse.bass_utils import run_bass_kernel_spmd

B, S, M, H, D = 2, 2048, 1024, 16, 64
HG = 4                 # heads per core
NCORES = 8
SQ = 512               # sq chunk width (one fp32 psum bank)
NSQ = S // SQ          # 4
NMC = M // 128         # 8 contraction chunks for projections
NSS = S // 128         # 16 s sub-chunks of 128

F8 = mybir.dt.float8e4
BF = mybir.dt.bfloat16
F32 = mybir.dt.float32
EXP = mybir.ActivationFunctionType.Exp
DR = mybir.MatmulPerfMode.DoubleRow

_f8 = ml_dtypes.float8_e4m3
_bf16 = ml_dtypes.bfloat16

# Schraudolph exp as a bf16 bit pattern: uint16(x * 128/ln2 + 16250)
# (~±3% relative; used on the DVE for half the last window's chunks, where
# the kernel is otherwise ScalarE-ACTIVATE bound)
EXPA = 184.6650390625
EXPB = 16250.0
U16 = mybir.dt.uint16


def _build_nc():
    nc = bacc.Bacc(
        "TRN2", target_bir_lowering=False, debug=False, num_devices=NCORES
    )

    xq = nc.declare_dram_parameter("xq_t8", [128, NSQ, NMC, SQ], F8, isOutput=False)
    xk = nc.declare_dram_parameter("xk_t8", [128, NSQ, NMC, SQ], F8, isOutput=False)
    xv = nc.declare_dram_parameter("xv_t8", [128, NSS, NMC, 128], F8, isOutput=False)
    wqd = nc.declare_dram_parameter("wq8", [128, 2 * NMC * 128], F8, isOutput=False)
    wkd = nc.declare_dram_parameter("wk8", [128, 2 * NMC * 128], F8, isOutput=False)
    wvd = nc.declare_dram_parameter("wv8", [128, NMC * HG * D], F8, isOutput=False)
    wo = nc.declare_dram_parameter("wo_bf", [128, 2 * M], BF, isOutput=False)
    bqk = nc.declare_dram_parameter("bqk", [128, 4], F32, isOutput=False)
    bv = nc.declare_dram_parameter("bv", [1, HG * D], F32, isOutput=False)
    out_p = nc.declare_dram_parameter("out_p", [S, M], BF, isOutput=True)

    with tile.TileContext(nc) as tc:
        with (
            tc.tile_pool(name="persist", bufs=1) as pers,
            tc.tile_pool(name="work", bufs=2) as work,
            tc.tile_pool(name="pps", bufs=3, space="PSUM") as pps,
            tc.tile_pool(name="ppz", bufs=2, space="PSUM") as ppz,
        ):
            # ---- persistent SBUF tensors ----
            xq_sb = pers.tile([128, NSQ, NMC, SQ], F8, tag="xq")
            xk_sb = pers.tile([128, NSQ, NMC, SQ], F8, tag="xk")
            xv_sb = pers.tile([128, NSS, NMC, 128], F8, tag="xv")
            wq_c = [pers.tile([128, NMC, 128], F8, tag=f"wq{c}",
                                 name=f"wq_c{c}") for c in range(2)]
            wk_c = [pers.tile([128, NMC, 128], F8, tag=f"wk{c}",
                                 name=f"wk_c{c}") for c in range(2)]
            wv_sb = pers.tile([128, NMC, HG * D], F8, tag="wv")
            wo_sb = pers.tile([128, 2, M], BF, tag="wo")
            bqk_sb = pers.tile([128, 4], F32, tag="bqk")
            bv_sb = pers.tile([1, HG * D], F32, tag="bv")
            bvb_sb = pers.tile([128, HG, D], F32, tag="bvb")
            qt_sb = pers.tile([128, 2, S], BF, tag="qt")
            kt_sb = pers.tile([128, 2, S], BF, tag="kt")
            zt_sb = pers.tile([128, 2, S], BF, tag="zt")
            v_sb = pers.tile([128, NSS, HG, D + 1], BF, tag="v")
            expwarm = pers.tile([1, 1], F32, tag="expwarm")

            # ---- input DMAs (issued before anything that could block the
            # issuing queues), then constants / exp-table warm-up ----
            wq4 = wqd[:, :].rearrange("p (c m d) -> p c m d", c=2, m=NMC)
            wk4 = wkd[:, :].rearrange("p (c m d) -> p c m d", c=2, m=NMC)
            wv4 = wvd[:, :].rearrange("p (m d) -> p m d", m=NMC)
            nc.scalar.dma_start(out=bqk_sb[:, :], in_=bqk[:, :])
            nc.scalar.dma_start(out=bv_sb[:, :], in_=bv[:, :])
            nc.scalar.dma_start(out=wq_c[0][:, :, :], in_=wq4[:, 0])
            nc.sync.dma_start(out=xq_sb[:, 0, 0:4], in_=xq[:, 0, 0:4])
            nc.scalar.dma_start(out=wk_c[0][:, :, :], in_=wk4[:, 0])
            nc.sync.dma_start(out=xq_sb[:, 0, 4:8], in_=xq[:, 0, 4:8])
            nc.scalar.dma_start(out=wv_sb[:, :, :], in_=wv4)
            nc.sync.dma_start(out=xk_sb[:, 0, 0:4], in_=xk[:, 0, 0:4])
            nc.scalar.dma_start(out=wq_c[1][:, :, :], in_=wq4[:, 1])
            nc.sync.dma_start(out=xk_sb[:, 0, 4:8], in_=xk[:, 0, 4:8])
            nc.scalar.dma_start(out=wk_c[1][:, :, :], in_=wk4[:, 1])
            nc.sync.dma_start(out=xv_sb[:, 0:2], in_=xv[:, 0:2])
            nc.sync.dma_start(out=xv_sb[:, 2:4], in_=xv[:, 2:4])
            for t in range(1, NSQ):
                nc.sync.dma_start(out=xq_sb[:, t], in_=xq[:, t])
                nc.sync.dma_start(out=xk_sb[:, t], in_=xk[:, t])
                nc.sync.dma_start(out=xv_sb[:, 4 * t : 4 * t + 4],
                                  in_=xv[:, 4 * t : 4 * t + 4])
            nc.scalar.dma_start(
                out=wo_sb[:, :, :], in_=wo[:, :].rearrange("p (c m) -> p c m", c=2)
            )

            # warm the exp table set while the DMAs run
            nc.gpsimd.memset(expwarm[:, :], 1.0)
            nc.gpsimd.memset(v_sb[:, :, :, D : D + 1], 1.0)
            nc.scalar.activation(expwarm[:, :], expwarm[:, :], EXP)

            # keep the PE HAM activity monitor busy while the first DMAs
            # land so the real matmuls start at the warm (2.4 GHz) clock
            warm_bf = pers.tile([1, 64], BF, tag="warm_bf")
            nc.gpsimd.memset(warm_bf[:, :], 1.0)

            # broadcast b_V across partitions for the v-eviction add
            nc.gpsimd.partition_broadcast(
                bvb_sb.rearrange("p g d -> p (g d)"), bv_sb[0:1, :]
            )

            ps_w2 = pps.tile([128, 2, SQ], F32, tag="pps", name="ps_warm")
            for _ in range(36):
                nc.tensor.matmul(
                    ps_w2[0:64, 0, 0:64],
                    lhsT=warm_bf[0:1, :],
                    rhs=warm_bf[0:1, :],
                    start=True,
                    stop=True,
                )

            # ---- filler units (emitted inside the attention loop) ----
            def projqk_unit(dst_sb, w_sb, x_sb, bcol, t, c, scale=None):
                """qT/kT projection for s-window t, head-pair half c."""

                def emit():
                    ssl = slice(SQ * t, SQ * t + SQ)
                    ps2 = pps.tile([128, 2, SQ], F32, tag="pps", name=f"pj{t}_{c}")
                    ps = ps2[:, 0, :]
                    for mi in range(0, NMC, 2):
                        nc.tensor.matmul(
                            ps[:, :],
                            lhsT=w_sb[c][:, mi : mi + 2, :],
                            rhs=x_sb[:, t, mi : mi + 2, :],
                            start=(mi == 0),
                            stop=(mi == NMC - 2),
                            perf_mode=DR,
                        )
                    if scale is None:
                        nc.vector.tensor_scalar_add(
                            dst_sb[:, c, ssl], ps[:, :], bqk_sb[:, bcol : bcol + 1]
                        )
                    else:
                        # fold the 1/8 attention scale into the kT eviction
                        nc.vector.tensor_scalar(
                            out=dst_sb[:, c, ssl],
                            in0=ps[:, :],
                            scalar1=bqk_sb[:, bcol : bcol + 1],
                            scalar2=scale,
                            op0=mybir.AluOpType.add,
                            op1=mybir.AluOpType.mult,
                        )

                return emit

            def projv_unit(ss):
                """v projection for s-subchunk ss (128 rows)."""

                def emit():
                    psl = slice(128 * ss, 128 * ss + 128)
                    ps2 = pps.tile([128, 2, SQ], F32, tag="pps", name=f"pv{ss}")
                    ps = ps2[:, 0, :]
                    for mi in range(0, NMC, 2):
                        nc.tensor.matmul(
                            ps[:, 0 : HG * D],
                            lhsT=xv_sb[:, ss, mi : mi + 2, :],
                            rhs=wv_sb[:, mi : mi + 2, :],
                            start=(mi == 0),
                            stop=(mi == NMC - 2),
                            perf_mode=DR,
                        )
                    nc.vector.tensor_tensor(
                        out=v_sb[:, ss, :, 0:D],
                        in0=ps[:, 0 : HG * D].rearrange("p (g d) -> p g d", g=HG),
                        in1=bvb_sb[:, :, :],
                        op=mybir.AluOpType.add,
                    )

                return emit

            def outproj_unit(jq, ss4, n, evict=None):
                """output projection for rows [512*jq + 128*ss4, +128),
                columns [512*n, +512)."""

                def emit():
                    psl = slice(SQ * jq + 128 * ss4, SQ * jq + 128 * ss4 + 128)
                    nsl = slice(SQ * n, SQ * n + SQ)
                    o16 = work.tile([128, SQ], BF, tag="o16", bufs=3,
                                    name=f"o{jq}_{ss4}_{n}")
                    ps_o2 = pps.tile([128, 2, SQ], F32, tag="pps",
                                     name=f"po{jq}_{ss4}_{n}")
                    ps_o = ps_o2[:, 0, :]
                    for c in range(2):
                        nc.tensor.matmul(
                            ps_o[:, :],
                            lhsT=zt_sb[:, c, psl],
                            rhs=wo_sb[:, c, nsl],
                            start=(c == 0),
                            stop=(c == 1),
                        )
                    if evict == "scalar":
                        nc.scalar.copy(o16[:, :], ps_o[:, :])
                    else:
                        nc.vector.tensor_copy(o16[:, :], ps_o[:, :])
                    nc.sync.dma_start(out=out_p[psl, nsl], in_=o16[:, :])

                return emit

            def emit_z(ps_z, c, prev, jq, last, only_u=None):
                """z += v.T @ pattern for one sk-chunk. On diagonal-band
                chunks the causally-full columns go first (they do not wait
                for the gpsimd mask); the masked diagonal block follows."""
                pp, psi, pw0 = prev
                band = psi >= 4 * jq
                ranges = []
                if band and pw0 + 128 < SQ:
                    ranges.append((pw0 + 128, SQ, False))
                if band:
                    ranges.append((pw0, pw0 + 128, True))
                if not band:
                    ranges.append((pw0, SQ, False))
                for u in range(2) if only_u is None else (only_u,):
                    for ri, (lo, hi, _) in enumerate(ranges):
                        nc.tensor.matmul(
                            ps_z[u][:, lo:hi],
                            lhsT=v_sb[:, psi, 2 * c + u, :],
                            rhs=pp[:, u, lo:hi],
                            start=(psi == 0 and ri == 0),
                            stop=(last and ri == len(ranges) - 1),
                        )

            # ---- fused schedule ----
            # minimal pre-loop: just q/k window 0 half 0 before jq=0
            projqk_unit(qt_sb, wq_c, xq_sb, 0, 0, 0)()
            projqk_unit(kt_sb, wk_c, xk_sb, 2, 0, 0, 0.125)()

            for jq in range(NSQ):
                qsl = slice(SQ * jq, SQ * jq + SQ)
                nsk = 4 * (jq + 1)
                # deadline-driven filler assignment:
                #   c0 slot i (i<4): v chunk 4*jq+i (first used by z at
                #     absolute chunk 4*jq+i, i.e. c0 slot 4*jq+i+1)
                #   c0 later slots + c1 slots: next window's q/k and the
                #     previous window's output projection
                fill = {0: {}, 1: {}}
                for i in range(4):
                    fill[0].setdefault(i, []).append(projv_unit(4 * jq + i))
                if jq == 0:
                    fill[0].setdefault(1, []).append(
                        projqk_unit(qt_sb, wq_c, xq_sb, 1, 0, 1))
                    fill[0].setdefault(2, []).append(
                        projqk_unit(kt_sb, wk_c, xk_sb, 3, 0, 1, 0.125))
                rest0, rest1 = [], []
                if jq > 0:
                    oph = [outproj_unit(jq - 1, ss4, n)
                           for ss4 in range(4) for n in range(2)]
                    rest0 += oph[:4]
                    rest1 += oph[4:]
                if jq < NSQ - 1:
                    t = jq + 1
                    rest1 = [
                        projqk_unit(qt_sb, wq_c, xq_sb, 0, t, 0),
                        projqk_unit(kt_sb, wk_c, xk_sb, 2, t, 0, 0.125),
                        projqk_unit(qt_sb, wq_c, xq_sb, 1, t, 1),
                        projqk_unit(kt_sb, wk_c, xk_sb, 3, t, 1, 0.125),
                    ] + rest1
                for lst, cc in ((rest0, 0), (rest1, 1)):
                    free = nsk - 4 if cc == 0 else nsk
                    base = 4 if cc == 0 else 0
                    for i, u in enumerate(lst):
                        s = base + (i * free // len(lst) if free > 0 else i)
                        fill[cc].setdefault(min(s, max(nsk - 2, 0)), []).append(u)

                for c in range(2):  # head pair: heads (2c, 2c+1)
                    ps_z = [
                        ppz.tile([D + 1, SQ], F32, tag="ppz", name=f"psz{jq}_{c}_{u}")
                        for u in range(2)
                    ]
                    prev = None  # delayed-z pipeline: (p_bf, si, w0)
                    for si in range(nsk):
                        ksl = slice(128 * si, 128 * si + 128)
                        r = si - 4 * jq  # >=0 on diagonal-band tiles
                        w0 = 128 * r if r > 0 else 0  # fully-masked prefix
                        # both heads' scores into one 2-bank psum tile
                        ps2 = pps.tile([128, 2, SQ], F32, tag="pps",
                                       name=f"ps{jq}_{c}_{si}")
                        for u in range(2):
                            hsl = slice(64 * u, 64 * u + 64)
                            nc.tensor.matmul(
                                ps2[:, u, w0:SQ],
                                lhsT=kt_sb[hsl, c, ksl],
                                rhs=qt_sb[hsl, c, SQ * jq + w0 : SQ * jq + SQ],
                                start=True,
                                stop=True,
                            )
                        p_bf = work.tile([128, 2, SQ], BF, tag="p", bufs=4,
                                         name=f"p{jq}_{c}_{si}")
                        nc.scalar.activation(
                            p_bf[:, :, w0:SQ], ps2[:, :, w0:SQ], EXP
                        )
                        if r >= 0:
                            # in-place triangular mask on the diagonal block,
                            # both heads in one gpsimd op: keep col >= row
                            nc.gpsimd.affine_select(
                                out=p_bf[:, :, w0 : w0 + 128],
                                in_=p_bf[:, :, w0 : w0 + 128],
                                compare_op=mybir.AluOpType.is_ge,
                                fill=0.0,
                                base=0,
                                pattern=[[0, 2], [1, 128]],
                                channel_multiplier=-1,
                            )
                        if prev is not None:
                            emit_z(ps_z, c, prev, jq, last=False)
                        prev = (p_bf, si, w0)
                        for emit in fill[c].get(si, ()):
                            emit()
                    # drain the pipelined z for the last sk-chunk
                    emit_z(ps_z, c, prev, jq, last=True)
                    # normalize both heads of the pair.  The z psum is freed
                    # as fast as possible: the denominator row goes out via a
                    # ScalarE copy while a DVE copy evicts the raw z rows; the
                    # recip/broadcast/multiply then run off the critical path.
                    recips, rbs, zraws = [], [], []
                    dns = []
                    for u in range(2):
                        dn = work.tile([1, SQ], F32, tag="dn", name=f"dn{jq}{c}{u}")
                        nc.vector.tensor_copy(dn[:, :], ps_z[u][D : D + 1, :])
                        dns.append(dn)
                    for u in range(2):
                        recip = work.tile([1, SQ], F32, tag="recip",
                                          name=f"rc{jq}{c}{u}")
                        nc.vector.reciprocal_approx_fast(
                            out=recip[:, :], in_=dns[u][:, :]
                        )
                        recips.append(recip)
                    for u in range(2):
                        rb = work.tile([D, SQ], F32, tag="rb", name=f"rb{jq}{c}{u}")
                        nc.gpsimd.partition_broadcast(rb[:, :], recips[u][0:1, :])
                        rbs.append(rb)
                    if jq == NSQ - 1 and c == 1:
                        # fine-grained tail: normalize per 128-row slice and
                        # start that slice's output projection immediately
                        for ss4 in range(4):
                            fsl = slice(128 * ss4, 128 * ss4 + 128)
                            for u in range(2):
                                nc.vector.tensor_mul(
                                    zt_sb[64 * u : 64 * u + 64, c,
                                          SQ * jq + 128 * ss4 : SQ * jq + 128 * ss4 + 128],
                                    ps_z[u][0:D, fsl],
                                    rbs[u][:, fsl],
                                )
                            outproj_unit(jq, ss4, 0, evict="scalar")()
                            outproj_unit(jq, ss4, 1)()
                    else:
                        for u in range(2):
                            nc.vector.tensor_mul(
                                zt_sb[64 * u : 64 * u + 64, c, qsl],
                                ps_z[u][0:D, :],
                                rbs[u][:, :],
                            )

    if not nc.is_finalized():
        nc.finalize()
    return nc


_NC = None


def _get_nc():
    global _NC
    if _NC is None:
        _NC = _build_nc()
    return _NC


def _wpack(w):
    """[M, HG*D] -> partition-major [128, NMC*HG*D] (2 KiB contiguous rows)."""
    return np.ascontiguousarray(
        w.reshape(NMC, 128, HG * D).transpose(1, 0, 2).reshape(128, NMC * HG * D)
    )


def _wpack_c(w):
    """[M, HG*D] -> [128, 2(c-half), NMC, 128] flattened (c-half contiguous)."""
    return np.ascontiguousarray(
        w.reshape(NMC, 128, 2, 128).transpose(1, 2, 0, 3).reshape(128, 2 * NMC * 128)
    )


def _make_in_maps(inputs):
    q8 = lambda a: np.asarray(a, np.float32).astype(_f8)
    xt = {}
    for name, key in (("xq_t8", "query_input"), ("xk_t8", "key_input"),
                      ("xv_t8", "value_input")):
        # [S, M] -> fp8 [M, S] -> [p=128, t=4, mi=8, s'=512] (SBUF layout)
        if name == "xv_t8":
            # subchunk-major: [p, ss=16, mi=8, 128] so the v projection for
            # s-subchunk ss gates on a 128 KiB slice instead of a full window
            xt[name] = [
                np.ascontiguousarray(
                    q8(inputs[key][b]).T.reshape(NMC, 128, NSS, 128)
                    .transpose(1, 2, 0, 3)
                )
                for b in range(B)
            ]
        else:
            xt[name] = [
                np.ascontiguousarray(
                    q8(inputs[key][b]).T.reshape(NMC, 128, NSQ, SQ)
                    .transpose(1, 2, 0, 3)
                )
                for b in range(B)
            ]

    wq8 = q8(inputs["W_Q"])  # [H, M, D]
    wk8 = q8(inputs["W_K"])
    wv8 = q8(inputs["W_V"])
    wo = np.asarray(inputs["W_O"], np.float32)  # [H, D, M]

    in_maps = []
    for core in range(NCORES):
        b, hg = core // HG, core % HG
        hs = slice(HG * hg, HG * hg + HG)
        m = {
            "xq_t8": xt["xq_t8"][b],
            "xk_t8": xt["xk_t8"][b],
            "xv_t8": xt["xv_t8"][b],
            "wq8": _wpack_c(wq8[hs].transpose(1, 0, 2).reshape(M, HG * D)),
            "wk8": _wpack_c(wk8[hs].transpose(1, 0, 2).reshape(M, HG * D)),
            "wv8": _wpack(wv8[hs].transpose(1, 0, 2).reshape(M, HG * D)),
            "wo_bf": np.ascontiguousarray(
                wo[hs]
                .reshape(HG * D, M)
                .astype(_bf16)
                .reshape(2, 128, M)
                .transpose(1, 0, 2)
                .reshape(128, 2 * M)
            ),
            "bqk": np.ascontiguousarray(
                np.concatenate(
                    [
                        np.asarray(inputs[k], np.float32)[hs].reshape(2, 128).T
                        for k in ("b_Q", "b_K")
                    ],
                    axis=1,
                )
            ),
            "bv": np.asarray(inputs["b_V"], np.float32)[hs].reshape(1, HG * D).copy(),
        }
        in_maps.append(m)
    return in_maps


def _run(inputs, **kw):
    nc = _get_nc()
    in_maps = _make_in_maps(inputs)
    res = run_bass_kernel_spmd(nc, in_maps, list(range(NCORES)), **kw)
    out = np.zeros((B, S, M), np.float32)
    for core in range(NCORES):
        out[core // HG] += res.results[core]["out_p"].astype(np.float32)
    out += np.asarray(inputs["b_O"], np.float32)
    return out, res


def kernel(**inputs):
    out, _ = _run(inputs)
    return out



# revision 55
# speedup vs baseline: 1.1658x; 1.1594x over previous
"""Trainium2 Bass/Tile kernel: fused fp8-quantized multi-head causal attention.

Module: q/k/v = fp8(x) @ fp8(W) + b ; scores = (q k^T)/sqrt(64) with causal
mask (-1000 => exp underflows to exactly 0) ; out = softmax(scores) @ v @ W_O + b_O.

Sharding (8 NeuronCores, SPMD, no collectives):
  core c -> batch b = c // 4, head group hg = c % 4 (heads 4*hg .. 4*hg+3).
  Each core returns a partial [S, M] bf16 output (its 4 heads' contribution);
  the host sums the 4 partials per batch (fp32) and adds b_O.

Host-side preprocessing: inputs/W_{Q,K,V} are quantized to fp8-e4m3 on the
host (bit-identical to the reference's jnp e4m3fn cast for |x| <= 240) and
uploaded in the exact SBUF layout so every DMA is fully contiguous.
Startup-latency layout choices (the head is DMA-bandwidth-bound):
  * W_Q/W_K are packed [128, 2 c-halves, 8 m-chunks, 128] and DMA'd per
    c-half (128 KiB), so the first q/k projections wait only on their own
    half; W_V keeps the [128, 8, 256] layout (the v-proj rhs needs a 3D AP).
  * x_v is packed subchunk-major [128, 16 s-subchunks, 8 m-chunks, 128] so
    the first v projection gates on a 256 KiB slice instead of a full
    512 KiB window; xq/xk window-0 DMAs are split into two m-halves so the
    projection matmuls pipeline behind the transfers (view-level deps).
  This moves the first real matmul from ~18 us to ~14 us after kernel start.

On-chip dataflow per core (single fused loop, paced by ScalarE exp and the
PE roughly equally):
  qT, kT   : [d'=256, S] bf16, fp8 DoubleRow matmuls; bias added in the DVE
             psum eviction; the 1/8 score scale is folded into the kT
             eviction (fused add+mult tensor_scalar).
  v        : [S, 4x(64+ones-col)] bf16 (ones col accumulates the softmax
             denominator in z's row 64 for free); b_V is added by the DVE
             eviction against a gpsimd-broadcast bias tile (no fp32 matmuls).
  scores^T : one 2-bank psum [sk=128, 2, sq=512] per sk-chunk holds BOTH
             heads of an even/odd pair (row-group-packed concurrent matmuls,
             K=64 each). Diagonal-band tiles compute only the causally-live
             column range. pps bufs=3 lets scores run two exps ahead.
  pattern  : ONE exp per sk-chunk on ScalarE over both heads (2x512-wide)
             -> bf16; the diagonal [128,2,128] triangular mask is applied
             in-place by a single gpsimd affine_select covering both heads.
  z^T+denom: psum [65, sq] += v_h(lhsT [sk,65]) @ pattern, software-pipelined
             one sk-chunk BEHIND the scores/exp so the PE never stalls on
             the scalar engine. On band chunks the causally-full columns are
             a separate matmul that does not wait for the gpsimd mask.
  normalize: 1/denom via reciprocal_approx_fast, gpsimd partition_broadcast,
             DVE multiply -> bf16 zT.
  out      : psum [s=128, m=512] = zt(lhsT [hd=128,s])^T @ W_O; evicted to
             bf16 ([128,512] halves) and DMA'd per half.
  Scheduling: the qT/kT/v projections and the previous window's output
  projection are emitted as deadline-driven filler units inside the
  attention loop (one per sk-chunk slot), so DMA-in, projections, attention
  and DMA-out all overlap. A block of 36 tiny matmuls right after the DMA
  issues keeps the PE HAM activity monitor warm so real matmuls start at
  2.4 GHz; a few wide (N=512) warmups after them bridge the remaining
  DMA-bound gap so the HAM never re-throttles before the first projection.
  The last window's normalize+output-projection tail is
  fine-grained per 128-row slice with psum evictions moved to the by-then
  idle scalar engine.
"""

import os
import sys

for _p in ("/opt/trn_rl_repo", os.path.expanduser("~/.axon_site/_ro/trn_rl_repo")):
    if os.path.isdir(_p) and _p not in sys.path:
        sys.path.insert(0, _p)

import ml_dtypes
import numpy as np

import concourse.bass as bass
import concourse.mybir as mybir
import concourse.tile as tile
from concourse import bacc
from concourse.bass_utils import run_bass_kernel_spmd

B, S, M, H, D = 2, 2048, 1024, 16, 64
HG = 4                 # heads per core
NCORES = 8
SQ = 512               # sq chunk width (one fp32 psum bank)
NSQ = S // SQ          # 4
NMC = M // 128         # 8 contraction chunks for projections
NSS = S // 128         # 16 s sub-chunks of 128

F8 = mybir.dt.float8e4
BF = mybir.dt.bfloat16
F32 = mybir.dt.float32
EXP = mybir.ActivationFunctionType.Exp
DR = mybir.MatmulPerfMode.DoubleRow

_f8 = ml_dtypes.float8_e4m3
_bf16 = ml_dtypes.bfloat16

# Schraudolph exp as a bf16 bit pattern: uint16(x * 128/ln2 + 16250)
# (~±3% relative; used on the DVE for half the last window's chunks, where
# the kernel is otherwise ScalarE-ACTIVATE bound)
EXPA = 184.6650390625
EXPB = 16250.0
U16 = mybir.dt.uint16


def _build_nc():
    nc = bacc.Bacc(
        "TRN2", target_bir_lowering=False, debug=False, num_devices=NCORES
    )

    xq = nc.declare_dram_parameter("xq_t8", [128, NSQ, NMC, SQ], F8, isOutput=False)
    xk = nc.declare_dram_parameter("xk_t8", [128, NSQ, NMC, SQ], F8, isOutput=False)
    xv = nc.declare_dram_parameter("xv_t8", [128, NSS, NMC, 128], F8, isOutput=False)
    wqd = nc.declare_dram_parameter("wq8", [128, 2 * NMC * 128], F8, isOutput=False)
    wkd = nc.declare_dram_parameter("wk8", [128, 2 * NMC * 128], F8, isOutput=False)
    wvd = nc.declare_dram_parameter("wv8", [128, NMC * HG * D], F8, isOutput=False)
    wo = nc.declare_dram_parameter("wo_bf", [128, 2 * M], BF, isOutput=False)
    bqk = nc.declare_dram_parameter("bqk", [128, 4], F32, isOutput=False)
    bv = nc.declare_dram_parameter("bv", [1, HG * D], F32, isOutput=False)
    out_p = nc.declare_dram_parameter("out_p", [S, M], BF, isOutput=True)

    with tile.TileContext(nc) as tc:
        with (
            tc.tile_pool(name="persist", bufs=1) as pers,
            tc.tile_pool(name="work", bufs=2) as work,
            tc.tile_pool(name="pps", bufs=3, space="PSUM") as pps,
            tc.tile_pool(name="ppz", bufs=2, space="PSUM") as ppz,
        ):
            # ---- persistent SBUF tensors ----
            xq_sb = pers.tile([128, NSQ, NMC, SQ], F8, tag="xq")
            xk_sb = pers.tile([128, NSQ, NMC, SQ], F8, tag="xk")
            xv_sb = pers.tile([128, NSS, NMC, 128], F8, tag="xv")
            wq_c = [pers.tile([128, NMC, 128], F8, tag=f"wq{c}",
                                 name=f"wq_c{c}") for c in range(2)]
            wk_c = [pers.tile([128, NMC, 128], F8, tag=f"wk{c}",
                                 name=f"wk_c{c}") for c in range(2)]
            wv_sb = pers.tile([128, NMC, HG * D], F8, tag="wv")
            wo_sb = pers.tile([128, 2, M], BF, tag="wo")
            bqk_sb = pers.tile([128, 4], F32, tag="bqk")
            bv_sb = pers.tile([1, HG * D], F32, tag="bv")
            bvb_sb = pers.tile([128, HG, D], F32, tag="bvb")
            qt_sb = pers.tile([128, 2, S], BF, tag="qt")
            kt_sb = pers.tile([128, 2, S], BF, tag="kt")
            zt_sb = pers.tile([128, 2, S], BF, tag="zt")
            v_sb = pers.tile([128, NSS, HG, D + 1], BF, tag="v")
            expwarm = pers.tile([1, 1], F32, tag="expwarm")

            # ---- input DMAs (issued before anything that could block the
            # issuing queues), then constants / exp-table warm-up ----
            wq4 = wqd[:, :].rearrange("p (c m d) -> p c m d", c=2, m=NMC)
            wk4 = wkd[:, :].rearrange("p (c m d) -> p c m d", c=2, m=NMC)
            wv4 = wvd[:, :].rearrange("p (m d) -> p m d", m=NMC)
            nc.scalar.dma_start(out=bqk_sb[:, :], in_=bqk[:, :])
            nc.scalar.dma_start(out=bv_sb[:, :], in_=bv[:, :])
            nc.scalar.dma_start(out=wq_c[0][:, :, :], in_=wq4[:, 0])
            nc.sync.dma_start(out=xq_sb[:, 0, 0:4], in_=xq[:, 0, 0:4])
            nc.scalar.dma_start(out=wk_c[0][:, :, :], in_=wk4[:, 0])
            nc.sync.dma_start(out=xq_sb[:, 0, 4:8], in_=xq[:, 0, 4:8])
            nc.scalar.dma_start(out=wv_sb[:, :, :], in_=wv4)
            nc.sync.dma_start(out=xk_sb[:, 0, 0:4], in_=xk[:, 0, 0:4])
            nc.scalar.dma_start(out=wq_c[1][:, :, :], in_=wq4[:, 1])
            nc.sync.dma_start(out=xk_sb[:, 0, 4:8], in_=xk[:, 0, 4:8])
            nc.scalar.dma_start(out=wk_c[1][:, :, :], in_=wk4[:, 1])
            nc.sync.dma_start(out=xv_sb[:, 0:2], in_=xv[:, 0:2])
            nc.sync.dma_start(out=xv_sb[:, 2:4], in_=xv[:, 2:4])
            for t in range(1, NSQ):
                nc.sync.dma_start(out=xq_sb[:, t], in_=xq[:, t])
                nc.sync.dma_start(out=xk_sb[:, t], in_=xk[:, t])
                nc.sync.dma_start(out=xv_sb[:, 4 * t : 4 * t + 4],
                                  in_=xv[:, 4 * t : 4 * t + 4])
            nc.scalar.dma_start(
                out=wo_sb[:, :, :], in_=wo[:, :].rearrange("p (c m) -> p c m", c=2)
            )

            # warm the exp table set while the DMAs run
            nc.gpsimd.memset(expwarm[:, :], 1.0)
            nc.gpsimd.memset(v_sb[:, :, :, D : D + 1], 1.0)
            nc.scalar.activation(expwarm[:, :], expwarm[:, :], EXP)

            # keep the PE HAM activity monitor busy while the first DMAs
            # land so the real matmuls start at the warm (2.4 GHz) clock
            warm_bf = pers.tile([1, 64], BF, tag="warm_bf")
            nc.gpsimd.memset(warm_bf[:, :], 1.0)
            warm_w = pers.tile([1, SQ], BF, tag="warm_w")
            nc.gpsimd.memset(warm_w[:, :], 1.0)

            # broadcast b_V across partitions for the v-eviction add
            nc.gpsimd.partition_broadcast(
                bvb_sb.rearrange("p g d -> p (g d)"), bv_sb[0:1, :]
            )

            ps_w2 = pps.tile([128, 2, SQ], F32, tag="pps", name="ps_warm")
            for _ in range(36):
                nc.tensor.matmul(
                    ps_w2[0:64, 0, 0:64],
                    lhsT=warm_bf[0:1, :],
                    rhs=warm_bf[0:1, :],
                    start=True,
                    stop=True,
                )
            # a few wide warmups extend PE activity past the DMA-bound gap
            # before the first projection, so the HAM MID window (~3.4 us of
            # idle) never re-throttles the clock to 1.2 GHz before real work
            for _ in range(6):
                nc.tensor.matmul(
                    ps_w2[0:64, 0, :],
                    lhsT=warm_bf[0:1, :],
                    rhs=warm_w[0:1, :],
                    start=True,
                    stop=True,
                )

            # ---- filler units (emitted inside the attention loop) ----
            def projqk_unit(dst_sb, w_sb, x_sb, bcol, t, c, scale=None):
                """qT/kT projection for s-window t, head-pair half c."""

                def emit():
                    ssl = slice(SQ * t, SQ * t + SQ)
                    ps2 = pps.tile([128, 2, SQ], F32, tag="pps", name=f"pj{t}_{c}")
                    ps = ps2[:, 0, :]
                    for mi in range(0, NMC, 2):
                        nc.tensor.matmul(
                            ps[:, :],
                            lhsT=w_sb[c][:, mi : mi + 2, :],
                            rhs=x_sb[:, t, mi : mi + 2, :],
                            start=(mi == 0),
                            stop=(mi == NMC - 2),
                            perf_mode=DR,
                        )
                    if scale is None:
                        nc.vector.tensor_scalar_add(
                            dst_sb[:, c, ssl], ps[:, :], bqk_sb[:, bcol : bcol + 1]
                        )
                    else:
                        # fold the 1/8 attention scale into the kT eviction
                        nc.vector.tensor_scalar(
                            out=dst_sb[:, c, ssl],
                            in0=ps[:, :],
                            scalar1=bqk_sb[:, bcol : bcol + 1],
                            scalar2=scale,
                            op0=mybir.AluOpType.add,
                            op1=mybir.AluOpType.mult,
                        )

                return emit

            def projv_unit(ss):
                """v projection for s-subchunk ss (128 rows)."""

                def emit():
                    psl = slice(128 * ss, 128 * ss + 128)
                    ps2 = pps.tile([128, 2, SQ], F32, tag="pps", name=f"pv{ss}")
                    ps = ps2[:, 0, :]
                    for mi in range(0, NMC, 2):
                        nc.tensor.matmul(
                            ps[:, 0 : HG * D],
                            lhsT=xv_sb[:, ss, mi : mi + 2, :],
                            rhs=wv_sb[:, mi : mi + 2, :],
                            start=(mi == 0),
                            stop=(mi == NMC - 2),
                            perf_mode=DR,
                        )
                    nc.vector.tensor_tensor(
                        out=v_sb[:, ss, :, 0:D],
                        in0=ps[:, 0 : HG * D].rearrange("p (g d) -> p g d", g=HG),
                        in1=bvb_sb[:, :, :],
                        op=mybir.AluOpType.add,
                    )

                return emit

            def outproj_unit(jq, ss4, n, evict=None):
                """output projection for rows [512*jq + 128*ss4, +128),
                columns [512*n, +512)."""

                def emit():
                    psl = slice(SQ * jq + 128 * ss4, SQ * jq + 128 * ss4 + 128)
                    nsl = slice(SQ * n, SQ * n + SQ)
                    o16 = work.tile([128, SQ], BF, tag="o16", bufs=3,
                                    name=f"o{jq}_{ss4}_{n}")
                    ps_o2 = pps.tile([128, 2, SQ], F32, tag="pps",
                                     name=f"po{jq}_{ss4}_{n}")
                    ps_o = ps_o2[:, 0, :]
                    for c in range(2):
                        nc.tensor.matmul(
                            ps_o[:, :],
                            lhsT=zt_sb[:, c, psl],
                            rhs=wo_sb[:, c, nsl],
                            start=(c == 0),
                            stop=(c == 1),
                        )
                    if evict == "scalar":
                        nc.scalar.copy(o16[:, :], ps_o[:, :])
                    else:
                        nc.vector.tensor_copy(o16[:, :], ps_o[:, :])
                    nc.sync.dma_start(out=out_p[psl, nsl], in_=o16[:, :])

                return emit

            def emit_z(ps_z, c, prev, jq, last, only_u=None):
                """z += v.T @ pattern for one sk-chunk. On diagonal-band
                chunks the causally-full columns go first (they do not wait
                for the gpsimd mask); the masked diagonal block follows."""
                pp, psi, pw0 = prev
                band = psi >= 4 * jq
                ranges = []
                if band and pw0 + 128 < SQ:
                    ranges.append((pw0 + 128, SQ, False))
                if band:
                    ranges.append((pw0, pw0 + 128, True))
                if not band:
                    ranges.append((pw0, SQ, False))
                for u in range(2) if only_u is None else (only_u,):
                    for ri, (lo, hi, _) in enumerate(ranges):
                        nc.tensor.matmul(
                            ps_z[u][:, lo:hi],
                            lhsT=v_sb[:, psi, 2 * c + u, :],
                            rhs=pp[:, u, lo:hi],
                            start=(psi == 0 and ri == 0),
                            stop=(last and ri == len(ranges) - 1),
                        )

            # ---- fused schedule ----
            # minimal pre-loop: just q/k window 0 half 0 before jq=0
            projqk_unit(qt_sb, wq_c, xq_sb, 0, 0, 0)()
            projqk_unit(kt_sb, wk_c, xk_sb, 2, 0, 0, 0.125)()

            for jq in range(NSQ):
                qsl = slice(SQ * jq, SQ * jq + SQ)
                nsk = 4 * (jq + 1)
                # deadline-driven filler assignment:
                #   c0 slot i (i<4): v chunk 4*jq+i (first used by z at
                #     absolute chunk 4*jq+i, i.e. c0 slot 4*jq+i+1)
                #   c0 later slots + c1 slots: next window's q/k and the
                #     previous window's output projection
                fill = {0: {}, 1: {}}
                for i in range(4):
                    fill[0].setdefault(i, []).append(projv_unit(4 * jq + i))
                if jq == 0:
                    fill[0].setdefault(1, []).append(
                        projqk_unit(qt_sb, wq_c, xq_sb, 1, 0, 1))
                    fill[0].setdefault(2, []).append(
                        projqk_unit(kt_sb, wk_c, xk_sb, 3, 0, 1, 0.125))
                rest0, rest1 = [], []
                if jq > 0:
                    oph = [outproj_unit(jq - 1, ss4, n)
                           for ss4 in range(4) for n in range(2)]
                    rest0 += oph[:4]
                    rest1 += oph[4:]
                if jq < NSQ - 1:
                    t = jq + 1
                    rest1 = [
                        projqk_unit(qt_sb, wq_c, xq_sb, 0, t, 0),
                        projqk_unit(kt_sb, wk_c, xk_sb, 2, t, 0, 0.125),
                        projqk_unit(qt_sb, wq_c, xq_sb, 1, t, 1),
                        projqk_unit(kt_sb, wk_c, xk_sb, 3, t, 1, 0.125),
                    ] + rest1
                for lst, cc in ((rest0, 0), (rest1, 1)):
                    free = nsk - 4 if cc == 0 else nsk
                    base = 4 if cc == 0 else 0
                    for i, u in enumerate(lst):
                        s = base + (i * free // len(lst) if free > 0 else i)
                        fill[cc].setdefault(min(s, max(nsk - 2, 0)), []).append(u)

                for c in range(2):  # head pair: heads (2c, 2c+1)
                    ps_z = [
                        ppz.tile([D + 1, SQ], F32, tag="ppz", name=f"psz{jq}_{c}_{u}")
                        for u in range(2)
                    ]
                    prev = None  # delayed-z pipeline: (p_bf, si, w0)
                    for si in range(nsk):
                        ksl = slice(128 * si, 128 * si + 128)
                        r = si - 4 * jq  # >=0 on diagonal-band tiles
                        w0 = 128 * r if r > 0 else 0  # fully-masked prefix
                        # both heads' scores into one 2-bank psum tile
                        ps2 = pps.tile([128, 2, SQ], F32, tag="pps",
                                       name=f"ps{jq}_{c}_{si}")
                        for u in range(2):
                            hsl = slice(64 * u, 64 * u + 64)
                            nc.tensor.matmul(
                                ps2[:, u, w0:SQ],
                                lhsT=kt_sb[hsl, c, ksl],
                                rhs=qt_sb[hsl, c, SQ * jq + w0 : SQ * jq + SQ],
                                start=True,
                                stop=True,
                            )
                        p_bf = work.tile([128, 2, SQ], BF, tag="p", bufs=4,
                                         name=f"p{jq}_{c}_{si}")
                        nc.scalar.activation(
                            p_bf[:, :, w0:SQ], ps2[:, :, w0:SQ], EXP
                        )
                        if r >= 0:
                            # in-place triangular mask on the diagonal block,
                            # both heads in one gpsimd op: keep col >= row
                            nc.gpsimd.affine_select(
                                out=p_bf[:, :, w0 : w0 + 128],
                                in_=p_bf[:, :, w0 : w0 + 128],
                                compare_op=mybir.AluOpType.is_ge,
                                fill=0.0,
                                base=0,
                                pattern=[[0, 2], [1, 128]],
                                channel_multiplier=-1,
                            )
                        if prev is not None:
                            emit_z(ps_z, c, prev, jq, last=False)
                        prev = (p_bf, si, w0)
                        for emit in fill[c].get(si, ()):
                            emit()
                    # drain the pipelined z for the last sk-chunk
                    emit_z(ps_z, c, prev, jq, last=True)
                    # normalize both heads of the pair.  The z psum is freed
                    # as fast as possible: the denominator row goes out via a
                    # ScalarE copy while a DVE copy evicts the raw z rows; the
                    # recip/broadcast/multiply then run off the critical path.
                    recips, rbs, zraws = [], [], []
                    dns = []
                    for u in range(2):
                        dn = work.tile([1, SQ], F32, tag="dn", name=f"dn{jq}{c}{u}")
                        nc.vector.tensor_copy(dn[:, :], ps_z[u][D : D + 1, :])
                        dns.append(dn)
                    for u in range(2):
                        recip = work.tile([1, SQ], F32, tag="recip",
                                          name=f"rc{jq}{c}{u}")
                        nc.vector.reciprocal_approx_fast(
                            out=recip[:, :], in_=dns[u][:, :]
                        )
                        recips.append(recip)
                    for u in range(2):
                        rb = work.tile([D, SQ], F32, tag="rb", name=f"rb{jq}{c}{u}")
                        nc.gpsimd.partition_broadcast(rb[:, :], recips[u][0:1, :])
                        rbs.append(rb)
                    if jq == NSQ - 1 and c == 1:
                        # fine-grained tail: normalize per 128-row slice and
                        # start that slice's output projection immediately
                        for ss4 in range(4):
                            fsl = slice(128 * ss4, 128 * ss4 + 128)
                            for u in range(2):
                                nc.vector.tensor_mul(
                                    zt_sb[64 * u : 64 * u + 64, c,
                                          SQ * jq + 128 * ss4 : SQ * jq + 128 * ss4 + 128],
                                    ps_z[u][0:D, fsl],
                                    rbs[u][:, fsl],
                                )
                            outproj_unit(jq, ss4, 0, evict="scalar")()
                            outproj_unit(jq, ss4, 1)()
                    else:
                        for u in range(2):
                            nc.vector.tensor_mul(
                                zt_sb[64 * u : 64 * u + 64, c, qsl],
                                ps_z[u][0:D, :],
                                rbs[u][:, :],
                            )

    if not nc.is_finalized():
        nc.finalize()
    return nc


_NC = None


def _get_nc():
    global _NC
    if _NC is None:
        _NC = _build_nc()
    return _NC


def _wpack(w):
    """[M, HG*D] -> partition-major [128, NMC*HG*D] (2 KiB contiguous rows)."""
    return np.ascontiguousarray(
        w.reshape(NMC, 128, HG * D).transpose(1, 0, 2).reshape(128, NMC * HG * D)
    )


def _wpack_c(w):
    """[M, HG*D] -> [128, 2(c-half), NMC, 128] flattened (c-half contiguous)."""
    return np.ascontiguousarray(
        w.reshape(NMC, 128, 2, 128).transpose(1, 2, 0, 3).reshape(128, 2 * NMC * 128)
    )


def _make_in_maps(inputs):
    q8 = lambda a: np.asarray(a, np.float32).astype(_f8)
    xt = {}
    for name, key in (("xq_t8", "query_input"), ("xk_t8", "key_input"),
                      ("xv_t8", "value_input")):
        # [S, M] -> fp8 [M, S] -> [p=128, t=4, mi=8, s'=512] (SBUF layout)
        if name == "xv_t8":
            # subchunk-major: [p, ss=16, mi=8, 128] so the v projection for
            # s-subchunk ss gates on a 128 KiB slice instead of a full window
            xt[name] = [
                np.ascontiguousarray(
                    q8(inputs[key][b]).T.reshape(NMC, 128, NSS, 128)
                    .transpose(1, 2, 0, 3)
                )
                for b in range(B)
            ]
        else:
            xt[name] = [
                np.ascontiguousarray(
                    q8(inputs[key][b]).T.reshape(NMC, 128, NSQ, SQ)
                    .transpose(1, 2, 0, 3)
                )
                for b in range(B)
            ]

    wq8 = q8(inputs["W_Q"])  # [H, M, D]
    wk8 = q8(inputs["W_K"])
    wv8 = q8(inputs["W_V"])
    wo = np.asarray(inputs["W_O"], np.float32)  # [H, D, M]

    in_maps = []
    for core in range(NCORES):
        b, hg = core // HG, core % HG
        hs = slice(HG * hg, HG * hg + HG)
        m = {
            "xq_t8": xt["xq_t8"][b],
            "xk_t8": xt["xk_t8"][b],
            "xv_t8": xt["xv_t8"][b],
            "wq8": _wpack_c(wq8[hs].transpose(1, 0, 2).reshape(M, HG * D)),
            "wk8": _wpack_c(wk8[hs].transpose(1, 0, 2).reshape(M, HG * D)),
            "wv8": _wpack(wv8[hs].transpose(1, 0, 2).reshape(M, HG * D)),
            "wo_bf": np.ascontiguousarray(
                wo[hs]
                .reshape(HG * D, M)
                .astype(_bf16)
                .reshape(2, 128, M)
                .transpose(1, 0, 2)
                .reshape(128, 2 * M)
            ),
            "bqk": np.ascontiguousarray(
                np.concatenate(
                    [
                        np.asarray(inputs[k], np.float32)[hs].reshape(2, 128).T
                        for k in ("b_Q", "b_K")
                    ],
                    axis=1,
                )
            ),
            "bv": np.asarray(inputs["b_V"], np.float32)[hs].reshape(1, HG * D).copy(),
        }
        in_maps.append(m)
    return in_maps


def _run(inputs, **kw):
    nc = _get_nc()
    in_maps = _make_in_maps(inputs)
    res = run_bass_kernel_spmd(nc, in_maps, list(range(NCORES)), **kw)
    out = np.zeros((B, S, M), np.float32)
    for core in range(NCORES):
        out[core // HG] += res.results[core]["out_p"].astype(np.float32)
    out += np.asarray(inputs["b_O"], np.float32)
    return out, res


def kernel(**inputs):
    out, _ = _run(inputs)
    return out



# revision 57
# speedup vs baseline: 1.1770x; 1.0096x over previous
"""Trainium2 Bass/Tile kernel: fused fp8-quantized multi-head causal attention.

Module: q/k/v = fp8(x) @ fp8(W) + b ; scores = (q k^T)/sqrt(64) with causal
mask (-1000 => exp underflows to exactly 0) ; out = softmax(scores) @ v @ W_O + b_O.

Sharding (8 NeuronCores, SPMD, no collectives):
  core c -> batch b = c // 4, head group hg = c % 4 (heads 4*hg .. 4*hg+3).
  Each core returns a partial [S, M] bf16 output (its 4 heads' contribution);
  the host sums the 4 partials per batch (fp32) and adds b_O.

Host-side preprocessing: inputs/W_{Q,K,V} are quantized to fp8-e4m3 on the
host (bit-identical to the reference's jnp e4m3fn cast for |x| <= 240) and
uploaded in the exact SBUF layout so every DMA is fully contiguous.
Startup-latency layout choices (the head is DMA-bandwidth-bound):
  * W_Q/W_K are packed [128, 2 c-halves, 8 m-chunks, 128] and DMA'd per
    c-half (128 KiB), so the first q/k projections wait only on their own
    half; W_V keeps the [128, 8, 256] layout (the v-proj rhs needs a 3D AP).
  * x_v is packed subchunk-major [128, 16 s-subchunks, 8 m-chunks, 128] so
    the first v projection gates on a 256 KiB slice instead of a full
    512 KiB window; xq/xk window-0 DMAs are split into two m-halves so the
    projection matmuls pipeline behind the transfers (view-level deps).
  This moves the first real matmul from ~18 us to ~14 us after kernel start.

On-chip dataflow per core (single fused loop, paced by ScalarE exp and the
PE roughly equally):
  qT, kT   : [d'=256, S] bf16, fp8 DoubleRow matmuls; bias added in the DVE
             psum eviction; the 1/8 score scale is folded into the kT
             eviction (fused add+mult tensor_scalar).
  v        : [S, 4x(64+ones-col)] bf16 (ones col accumulates the softmax
             denominator in z's row 64 for free); b_V is added by the DVE
             eviction against a gpsimd-broadcast bias tile (no fp32 matmuls).
  scores^T : one 2-bank psum [sk=128, 2, sq=512] per sk-chunk holds BOTH
             heads of an even/odd pair (row-group-packed concurrent matmuls,
             K=64 each). Diagonal-band tiles compute only the causally-live
             column range. pps bufs=3 lets scores run two exps ahead.
  pattern  : ONE exp per sk-chunk on ScalarE over both heads (2x512-wide)
             -> bf16; the diagonal [128,2,128] triangular mask is applied
             in-place by a single gpsimd affine_select covering both heads.
  z^T+denom: psum [65, sq] += v_h(lhsT [sk,65]) @ pattern, software-pipelined
             one sk-chunk BEHIND the scores/exp so the PE never stalls on
             the scalar engine. On band chunks the causally-full columns are
             a separate matmul that does not wait for the gpsimd mask.
  normalize: 1/denom via reciprocal_approx_fast, gpsimd partition_broadcast,
             DVE multiply -> bf16 zT.
  out      : psum [s=128, m=512] = zt(lhsT [hd=128,s])^T @ W_O; evicted to
             bf16 ([128,512] halves) and DMA'd per half.
  Scheduling: the qT/kT/v projections and the previous window's output
  projection are emitted as deadline-driven filler units inside the
  attention loop (one per sk-chunk slot), so DMA-in, projections, attention
  and DMA-out all overlap. A block of 36 tiny matmuls right after the DMA
  issues keeps the PE HAM activity monitor warm so real matmuls start at
  2.4 GHz; a few wide (N=512) warmups after them bridge the remaining
  DMA-bound gap so the HAM never re-throttles before the first projection.
  The last window's normalize+output-projection tail is
  fine-grained per 128-row slice with psum evictions moved to the by-then
  idle scalar engine.
"""

import os
import sys

for _p in ("/opt/trn_rl_repo", os.path.expanduser("~/.axon_site/_ro/trn_rl_repo")):
    if os.path.isdir(_p) and _p not in sys.path:
        sys.path.insert(0, _p)

import ml_dtypes
import numpy as np

import concourse.bass as bass
import concourse.mybir as mybir
import concourse.tile as tile
from concourse import bacc
from concourse.bass_utils import run_bass_kernel_spmd

B, S, M, H, D = 2, 2048, 1024, 16, 64
HG = 4                 # heads per core
NCORES = 8
SQ = 512               # sq chunk width (one fp32 psum bank)
NSQ = S // SQ          # 4
NMC = M // 128         # 8 contraction chunks for projections
NSS = S // 128         # 16 s sub-chunks of 128

F8 = mybir.dt.float8e4
BF = mybir.dt.bfloat16
F32 = mybir.dt.float32
EXP = mybir.ActivationFunctionType.Exp
DR = mybir.MatmulPerfMode.DoubleRow

_f8 = ml_dtypes.float8_e4m3
_bf16 = ml_dtypes.bfloat16

# Schraudolph exp as a bf16 bit pattern: uint16(x * 128/ln2 + 16250)
# (~±3% relative; used on the DVE for half the last window's chunks, where
# the kernel is otherwise ScalarE-ACTIVATE bound)
EXPA = 184.6650390625
EXPB = 16250.0
U16 = mybir.dt.uint16


def _build_nc():
    nc = bacc.Bacc(
        "TRN2", target_bir_lowering=False, debug=False, num_devices=NCORES
    )

    xq = nc.declare_dram_parameter("xq_t8", [128, NSQ, NMC, SQ], F8, isOutput=False)
    xk = nc.declare_dram_parameter("xk_t8", [128, NSQ, NMC, SQ], F8, isOutput=False)
    xv = nc.declare_dram_parameter("xv_t8", [128, NSS, NMC, 128], F8, isOutput=False)
    wqd = nc.declare_dram_parameter("wq8", [128, 2 * NMC * 128], F8, isOutput=False)
    wkd = nc.declare_dram_parameter("wk8", [128, 2 * NMC * 128], F8, isOutput=False)
    wvd = nc.declare_dram_parameter("wv8", [128, NMC * HG * D], F8, isOutput=False)
    wo = nc.declare_dram_parameter("wo_bf", [128, 2 * M], BF, isOutput=False)
    bqk = nc.declare_dram_parameter("bqk", [128, 4], F32, isOutput=False)
    bv = nc.declare_dram_parameter("bv", [1, HG * D], F32, isOutput=False)
    out_p = nc.declare_dram_parameter("out_p", [S, M], BF, isOutput=True)

    with tile.TileContext(nc) as tc:
        with (
            tc.tile_pool(name="persist", bufs=1) as pers,
            tc.tile_pool(name="work", bufs=2) as work,
            tc.tile_pool(name="pps", bufs=3, space="PSUM") as pps,
            tc.tile_pool(name="ppz", bufs=2, space="PSUM") as ppz,
        ):
            # ---- persistent SBUF tensors ----
            xq_sb = pers.tile([128, NSQ, NMC, SQ], F8, tag="xq")
            xk_sb = pers.tile([128, NSQ, NMC, SQ], F8, tag="xk")
            xv_sb = pers.tile([128, NSS, NMC, 128], F8, tag="xv")
            wq_c = [pers.tile([128, NMC, 128], F8, tag=f"wq{c}",
                                 name=f"wq_c{c}") for c in range(2)]
            wk_c = [pers.tile([128, NMC, 128], F8, tag=f"wk{c}",
                                 name=f"wk_c{c}") for c in range(2)]
            wv_sb = pers.tile([128, NMC, HG * D], F8, tag="wv")
            wo_sb = pers.tile([128, 2, M], BF, tag="wo")
            bqk_sb = pers.tile([128, 4], F32, tag="bqk")
            bv_sb = pers.tile([1, HG * D], F32, tag="bv")
            bvb_sb = pers.tile([128, HG, D], F32, tag="bvb")
            qt_sb = pers.tile([128, 2, S], BF, tag="qt")
            kt_sb = pers.tile([128, 2, S], BF, tag="kt")
            zt_sb = pers.tile([128, 2, S], BF, tag="zt")
            v_sb = pers.tile([128, NSS, HG, D + 1], BF, tag="v")
            expwarm = pers.tile([1, 1], F32, tag="expwarm")

            # ---- input DMAs (issued before anything that could block the
            # issuing queues), then constants / exp-table warm-up ----
            wq4 = wqd[:, :].rearrange("p (c m d) -> p c m d", c=2, m=NMC)
            wk4 = wkd[:, :].rearrange("p (c m d) -> p c m d", c=2, m=NMC)
            wv4 = wvd[:, :].rearrange("p (m d) -> p m d", m=NMC)
            nc.scalar.dma_start(out=bqk_sb[:, :], in_=bqk[:, :])
            nc.scalar.dma_start(out=bv_sb[:, :], in_=bv[:, :])
            nc.scalar.dma_start(out=wq_c[0][:, :, :], in_=wq4[:, 0])
            nc.sync.dma_start(out=xq_sb[:, 0, 0:4], in_=xq[:, 0, 0:4])
            nc.scalar.dma_start(out=wk_c[0][:, :, :], in_=wk4[:, 0])
            nc.sync.dma_start(out=xq_sb[:, 0, 4:8], in_=xq[:, 0, 4:8])
            nc.scalar.dma_start(out=wv_sb[:, :, :], in_=wv4)
            nc.sync.dma_start(out=xk_sb[:, 0, 0:4], in_=xk[:, 0, 0:4])
            nc.scalar.dma_start(out=wq_c[1][:, :, :], in_=wq4[:, 1])
            nc.sync.dma_start(out=xk_sb[:, 0, 4:8], in_=xk[:, 0, 4:8])
            nc.scalar.dma_start(out=wk_c[1][:, :, :], in_=wk4[:, 1])
            nc.sync.dma_start(out=xv_sb[:, 0:2], in_=xv[:, 0:2])
            nc.sync.dma_start(out=xv_sb[:, 2:4], in_=xv[:, 2:4])
            for t in range(1, NSQ):
                nc.sync.dma_start(out=xq_sb[:, t], in_=xq[:, t])
                nc.sync.dma_start(out=xk_sb[:, t], in_=xk[:, t])
                nc.sync.dma_start(out=xv_sb[:, 4 * t : 4 * t + 4],
                                  in_=xv[:, 4 * t : 4 * t + 4])
            nc.scalar.dma_start(
                out=wo_sb[:, :, :], in_=wo[:, :].rearrange("p (c m) -> p c m", c=2)
            )

            # warm the exp table set while the DMAs run
            nc.gpsimd.memset(expwarm[:, :], 1.0)
            nc.gpsimd.memset(v_sb[:, :, :, D : D + 1], 1.0)
            nc.scalar.activation(expwarm[:, :], expwarm[:, :], EXP)

            # keep the PE HAM activity monitor busy while the first DMAs
            # land so the real matmuls start at the warm (2.4 GHz) clock
            warm_bf = pers.tile([1, 64], BF, tag="warm_bf")
            nc.gpsimd.memset(warm_bf[:, :], 1.0)
            warm_w = pers.tile([1, SQ], BF, tag="warm_w")
            nc.gpsimd.memset(warm_w[:, :], 1.0)

            # broadcast b_V across partitions for the v-eviction add
            nc.gpsimd.partition_broadcast(
                bvb_sb.rearrange("p g d -> p (g d)"), bv_sb[0:1, :]
            )

            ps_w2 = pps.tile([128, 2, SQ], F32, tag="pps", name="ps_warm")
            for _ in range(36):
                nc.tensor.matmul(
                    ps_w2[0:64, 0, 0:64],
                    lhsT=warm_bf[0:1, :],
                    rhs=warm_bf[0:1, :],
                    start=True,
                    stop=True,
                )
            # a few wide warmups extend PE activity past the DMA-bound gap
            # before the first projection, so the HAM MID window (~3.4 us of
            # idle) never re-throttles the clock to 1.2 GHz before real work
            for _ in range(6):
                nc.tensor.matmul(
                    ps_w2[0:64, 0, :],
                    lhsT=warm_bf[0:1, :],
                    rhs=warm_w[0:1, :],
                    start=True,
                    stop=True,
                )

            # ---- filler units (emitted inside the attention loop) ----
            def projqk_unit(dst_sb, w_sb, x_sb, bcol, t, c, scale=None):
                """qT/kT projection for s-window t, head-pair half c."""

                def emit():
                    ssl = slice(SQ * t, SQ * t + SQ)
                    ps2 = pps.tile([128, 2, SQ], F32, tag="pps", name=f"pj{t}_{c}")
                    ps = ps2[:, 0, :]
                    for mi in range(0, NMC, 2):
                        nc.tensor.matmul(
                            ps[:, :],
                            lhsT=w_sb[c][:, mi : mi + 2, :],
                            rhs=x_sb[:, t, mi : mi + 2, :],
                            start=(mi == 0),
                            stop=(mi == NMC - 2),
                            perf_mode=DR,
                        )
                    if scale is None:
                        nc.vector.tensor_scalar_add(
                            dst_sb[:, c, ssl], ps[:, :], bqk_sb[:, bcol : bcol + 1]
                        )
                    else:
                        # fold the 1/8 attention scale into the kT eviction
                        nc.vector.tensor_scalar(
                            out=dst_sb[:, c, ssl],
                            in0=ps[:, :],
                            scalar1=bqk_sb[:, bcol : bcol + 1],
                            scalar2=scale,
                            op0=mybir.AluOpType.add,
                            op1=mybir.AluOpType.mult,
                        )

                return emit

            def projv_unit(ss):
                """v projection for s-subchunk ss (128 rows)."""

                def emit():
                    psl = slice(128 * ss, 128 * ss + 128)
                    ps2 = pps.tile([128, 2, SQ], F32, tag="pps", name=f"pv{ss}")
                    ps = ps2[:, 0, :]
                    for mi in range(0, NMC, 2):
                        nc.tensor.matmul(
                            ps[:, 0 : HG * D],
                            lhsT=xv_sb[:, ss, mi : mi + 2, :],
                            rhs=wv_sb[:, mi : mi + 2, :],
                            start=(mi == 0),
                            stop=(mi == NMC - 2),
                            perf_mode=DR,
                        )
                    nc.vector.tensor_tensor(
                        out=v_sb[:, ss, :, 0:D],
                        in0=ps[:, 0 : HG * D].rearrange("p (g d) -> p g d", g=HG),
                        in1=bvb_sb[:, :, :],
                        op=mybir.AluOpType.add,
                    )

                return emit

            def outproj_unit(jq, ss4, n, evict=None):
                """output projection for rows [512*jq + 128*ss4, +128),
                columns [512*n, +512)."""

                def emit():
                    psl = slice(SQ * jq + 128 * ss4, SQ * jq + 128 * ss4 + 128)
                    nsl = slice(SQ * n, SQ * n + SQ)
                    o16 = work.tile([128, SQ], BF, tag="o16", bufs=3,
                                    name=f"o{jq}_{ss4}_{n}")
                    ps_o2 = pps.tile([128, 2, SQ], F32, tag="pps",
                                     name=f"po{jq}_{ss4}_{n}")
                    ps_o = ps_o2[:, 0, :]
                    for c in range(2):
                        nc.tensor.matmul(
                            ps_o[:, :],
                            lhsT=zt_sb[:, c, psl],
                            rhs=wo_sb[:, c, nsl],
                            start=(c == 0),
                            stop=(c == 1),
                        )
                    if evict == "scalar":
                        nc.scalar.copy(o16[:, :], ps_o[:, :])
                    else:
                        nc.vector.tensor_copy(o16[:, :], ps_o[:, :])
                    nc.sync.dma_start(out=out_p[psl, nsl], in_=o16[:, :])

                return emit

            def emit_z(ps_z, c, prev, jq, last, only_u=None):
                """z += v.T @ pattern for one sk-chunk. On diagonal-band
                chunks the causally-full columns go first (they do not wait
                for the gpsimd mask); the masked diagonal block follows."""
                pp, psi, pw0 = prev
                band = psi >= 4 * jq
                ranges = []
                if band and pw0 + 128 < SQ:
                    ranges.append((pw0 + 128, SQ, False))
                if band:
                    ranges.append((pw0, pw0 + 128, True))
                if not band:
                    ranges.append((pw0, SQ, False))
                for u in range(2) if only_u is None else (only_u,):
                    for ri, (lo, hi, _) in enumerate(ranges):
                        nc.tensor.matmul(
                            ps_z[u][:, lo:hi],
                            lhsT=v_sb[:, psi, 2 * c + u, :],
                            rhs=pp[:, u, lo:hi],
                            start=(psi == 0 and ri == 0),
                            stop=(last and ri == len(ranges) - 1),
                        )

            # ---- fused schedule ----
            # minimal pre-loop: just q/k window 0 half 0 before jq=0
            projqk_unit(qt_sb, wq_c, xq_sb, 0, 0, 0)()
            projqk_unit(kt_sb, wk_c, xk_sb, 2, 0, 0, 0.125)()

            for jq in range(NSQ):
                qsl = slice(SQ * jq, SQ * jq + SQ)
                nsk = 4 * (jq + 1)
                # deadline-driven filler assignment:
                #   c0 slot i (i<4): v chunk 4*jq+i (first used by z at
                #     absolute chunk 4*jq+i, i.e. c0 slot 4*jq+i+1)
                #   c0 later slots + c1 slots: next window's q/k and the
                #     previous window's output projection
                fill = {0: {}, 1: {}}
                for i in range(4):
                    fill[0].setdefault(i, []).append(projv_unit(4 * jq + i))
                if jq == 0:
                    fill[0].setdefault(1, []).append(
                        projqk_unit(qt_sb, wq_c, xq_sb, 1, 0, 1))
                    fill[0].setdefault(2, []).append(
                        projqk_unit(kt_sb, wk_c, xk_sb, 3, 0, 1, 0.125))
                rest0, rest1 = [], []
                if jq > 0:
                    oph = [outproj_unit(jq - 1, ss4, n)
                           for ss4 in range(4) for n in range(2)]
                    rest0 += oph[:4]
                    rest1 += oph[4:]
                if jq < NSQ - 1:
                    t = jq + 1
                    rest1 = [
                        projqk_unit(qt_sb, wq_c, xq_sb, 0, t, 0),
                        projqk_unit(kt_sb, wk_c, xk_sb, 2, t, 0, 0.125),
                        projqk_unit(qt_sb, wq_c, xq_sb, 1, t, 1),
                        projqk_unit(kt_sb, wk_c, xk_sb, 3, t, 1, 0.125),
                    ] + rest1
                for lst, cc in ((rest0, 0), (rest1, 1)):
                    free = nsk - 4 if cc == 0 else nsk
                    base = 4 if cc == 0 else 0
                    for i, u in enumerate(lst):
                        s = base + (i * free // len(lst) if free > 0 else i)
                        fill[cc].setdefault(min(s, max(nsk - 2, 0)), []).append(u)

                for c in range(2):  # head pair: heads (2c, 2c+1)
                    ps_z = [
                        ppz.tile([D + 1, SQ], F32, tag="ppz", name=f"psz{jq}_{c}_{u}")
                        for u in range(2)
                    ]
                    prev = None  # delayed-z pipeline: (p_bf, si, w0)
                    for si in range(nsk):
                        ksl = slice(128 * si, 128 * si + 128)
                        r = si - 4 * jq  # >=0 on diagonal-band tiles
                        w0 = 128 * r if r > 0 else 0  # fully-masked prefix
                        # both heads' scores into one 2-bank psum tile
                        ps2 = pps.tile([128, 2, SQ], F32, tag="pps",
                                       name=f"ps{jq}_{c}_{si}")
                        for u in range(2):
                            hsl = slice(64 * u, 64 * u + 64)
                            nc.tensor.matmul(
                                ps2[:, u, w0:SQ],
                                lhsT=kt_sb[hsl, c, ksl],
                                rhs=qt_sb[hsl, c, SQ * jq + w0 : SQ * jq + SQ],
                                start=True,
                                stop=True,
                            )
                        p_bf = work.tile([128, 2, SQ], BF, tag="p", bufs=4,
                                         name=f"p{jq}_{c}_{si}")
                        nc.scalar.activation(
                            p_bf[:, :, w0:SQ], ps2[:, :, w0:SQ], EXP
                        )
                        if r >= 0:
                            # in-place triangular mask on the diagonal block,
                            # both heads in one gpsimd op: keep col >= row
                            nc.gpsimd.affine_select(
                                out=p_bf[:, :, w0 : w0 + 128],
                                in_=p_bf[:, :, w0 : w0 + 128],
                                compare_op=mybir.AluOpType.is_ge,
                                fill=0.0,
                                base=0,
                                pattern=[[0, 2], [1, 128]],
                                channel_multiplier=-1,
                            )
                        if prev is not None:
                            emit_z(ps_z, c, prev, jq, last=False)
                        prev = (p_bf, si, w0)
                        for emit in fill[c].get(si, ()):
                            emit()
                    # drain the pipelined z for the last sk-chunk
                    emit_z(ps_z, c, prev, jq, last=True)
                    # normalize both heads of the pair.  Emitted as complete
                    # per-head chains (dn -> recip -> broadcast) so u0's
                    # normalize multiply is not queued behind u1's recip in
                    # the DVE FIFO -- u0's z psum buffer frees earlier for
                    # the next head pair.
                    recips, rbs = [], []
                    for u in range(2):
                        dn = work.tile([1, SQ], F32, tag="dn", name=f"dn{jq}{c}{u}")
                        nc.vector.tensor_copy(dn[:, :], ps_z[u][D : D + 1, :])
                        recip = work.tile([1, SQ], F32, tag="recip",
                                          name=f"rc{jq}{c}{u}")
                        nc.vector.reciprocal_approx_fast(
                            out=recip[:, :], in_=dn[:, :]
                        )
                        recips.append(recip)
                        rb = work.tile([D, SQ], F32, tag="rb", name=f"rb{jq}{c}{u}")
                        nc.gpsimd.partition_broadcast(rb[:, :], recip[0:1, :])
                        rbs.append(rb)
                        if not (jq == NSQ - 1 and c == 1):
                            nc.vector.tensor_mul(
                                zt_sb[64 * u : 64 * u + 64, c, qsl],
                                ps_z[u][0:D, :],
                                rb[:, :],
                            )
                    if jq == NSQ - 1 and c == 1:
                        # fine-grained tail: normalize per 128-row slice and
                        # start that slice's output projection immediately
                        for ss4 in range(4):
                            fsl = slice(128 * ss4, 128 * ss4 + 128)
                            for u in range(2):
                                nc.vector.tensor_mul(
                                    zt_sb[64 * u : 64 * u + 64, c,
                                          SQ * jq + 128 * ss4 : SQ * jq + 128 * ss4 + 128],
                                    ps_z[u][0:D, fsl],
                                    rbs[u][:, fsl],
                                )
                            outproj_unit(jq, ss4, 0, evict="scalar")()
                            outproj_unit(jq, ss4, 1)()

    if not nc.is_finalized():
        nc.finalize()
    return nc


_NC = None


def _get_nc():
    global _NC
    if _NC is None:
        _NC = _build_nc()
    return _NC


def _wpack(w):
    """[M, HG*D] -> partition-major [128, NMC*HG*D] (2 KiB contiguous rows)."""
    return np.ascontiguousarray(
        w.reshape(NMC, 128, HG * D).transpose(1, 0, 2).reshape(128, NMC * HG * D)
    )


def _wpack_c(w):
    """[M, HG*D] -> [128, 2(c-half), NMC, 128] flattened (c-half contiguous)."""
    return np.ascontiguousarray(
        w.reshape(NMC, 128, 2, 128).transpose(1, 2, 0, 3).reshape(128, 2 * NMC * 128)
    )


def _make_in_maps(inputs):
    q8 = lambda a: np.asarray(a, np.float32).astype(_f8)
    xt = {}
    for name, key in (("xq_t8", "query_input"), ("xk_t8", "key_input"),
                      ("xv_t8", "value_input")):
        # [S, M] -> fp8 [M, S] -> [p=128, t=4, mi=8, s'=512] (SBUF layout)
        if name == "xv_t8":
            # subchunk-major: [p, ss=16, mi=8, 128] so the v projection for
            # s-subchunk ss gates on a 128 KiB slice instead of a full window
            xt[name] = [
                np.ascontiguousarray(
                    q8(inputs[key][b]).T.reshape(NMC, 128, NSS, 128)
                    .transpose(1, 2, 0, 3)
                )
                for b in range(B)
            ]
        else:
            xt[name] = [
                np.ascontiguousarray(
                    q8(inputs[key][b]).T.reshape(NMC, 128, NSQ, SQ)
                    .transpose(1, 2, 0, 3)
                )
                for b in range(B)
            ]

    wq8 = q8(inputs["W_Q"])  # [H, M, D]
    wk8 = q8(inputs["W_K"])
    wv8 = q8(inputs["W_V"])
    wo = np.asarray(inputs["W_O"], np.float32)  # [H, D, M]

    in_maps = []
    for core in range(NCORES):
        b, hg = core // HG, core % HG
        hs = slice(HG * hg, HG * hg + HG)
        m = {
            "xq_t8": xt["xq_t8"][b],
            "xk_t8": xt["xk_t8"][b],
            "xv_t8": xt["xv_t8"][b],
            "wq8": _wpack_c(wq8[hs].transpose(1, 0, 2).reshape(M, HG * D)),
            "wk8": _wpack_c(wk8[hs].transpose(1, 0, 2).reshape(M, HG * D)),
            "wv8": _wpack(wv8[hs].transpose(1, 0, 2).reshape(M, HG * D)),
            "wo_bf": np.ascontiguousarray(
                wo[hs]
                .reshape(HG * D, M)
                .astype(_bf16)
                .reshape(2, 128, M)
                .transpose(1, 0, 2)
                .reshape(128, 2 * M)
            ),
            "bqk": np.ascontiguousarray(
                np.concatenate(
                    [
                        np.asarray(inputs[k], np.float32)[hs].reshape(2, 128).T
                        for k in ("b_Q", "b_K")
                    ],
                    axis=1,
                )
            ),
            "bv": np.asarray(inputs["b_V"], np.float32)[hs].reshape(1, HG * D).copy(),
        }
        in_maps.append(m)
    return in_maps


def _run(inputs, **kw):
    nc = _get_nc()
    in_maps = _make_in_maps(inputs)
    res = run_bass_kernel_spmd(nc, in_maps, list(range(NCORES)), **kw)
    out = np.zeros((B, S, M), np.float32)
    for core in range(NCORES):
        out[core // HG] += res.results[core]["out_p"].astype(np.float32)
    out += np.asarray(inputs["b_O"], np.float32)
    return out, res


def kernel(**inputs):
    out, _ = _run(inputs)
    return out

